# revision 65
# baseline (speedup 1.0000x reference)
"""Trainium2 Bass kernel for nn_MultiHeadAttention (B=2,T=2048,D=1024,H=16,HD=64).

Sharding: 8 cores = 2 batches x 4 heads/core (tensor parallel over heads).
Each core computes q,k,v projections for its 4 heads, RoPE, causal
flash-attention, and a partial output projection (its heads' slice of Wp);
the host sums the 4 partials per batch.

v3 design (on top of the fully-pipelined v2), measured 223us -> ~194us:
  - Packed scores matmuls: q/k stored as head-PAIR tiles [128, T]; each
    pair's two 64-row scores matmuls run CONCURRENTLY in the two PE
    array row-group halves (tile_position (0,0)/(64,0)) into different
    psum banks — halves scores wall time.
  - Causal diagonal masking moved off the PE: exp output is multiplied
    by a precomputed 0/1 tile on the DVE (kills 64 mask matmuls + the
    u/l mask weights).
  - PE HAM warm-up: dummy matmuls during the initial DMA wait plus a
    2x dummy-LDWEIGHTS trickle per early iteration hold the PE clock at
    2.4 GHz through the fill phase.
  - Startup: input DMAs split into column chunks and priority-ordered
    across the three DMA queues (sync-HW, scalar-HW, gpsimd-SW) so the
    first projection matmul starts at ~10.5us instead of ~19us.
  - exp activation table prefetched with a dummy 1-element exp at t=0.
  - Chunk order [1,0,2,3,4,5,7,6]: chunk 0 (inputs already resident)
    fills the xT j=1 DMA window; small tail chain hangs off chunk 6
    with a fast-path epilogue (reciprocal+normalize read psum direct).
  - Zero-init matmuls for the PV accumulator removed: the first PV
    matmul of each psum bank uses start=True (clears the whole bank's
    has_written bits; the co-banked head's first matmul then overwrites
    since its bits are clear).
  - q/k RoPE'd tiles stored with lo/hi rows interleaved (one combined
    DMA per head instead of two): scores are invariant to any row
    permutation applied consistently to q and k.
  - Output stored bf16 (host accumulates partials in fp32); tail
    stores pipelined per 512-col half on the HW DGE queues.
"""

import os
import sys

sys.path.insert(0, "/opt/trn_rl_repo")

from contextlib import ExitStack

import numpy as np
import ml_dtypes

import concourse.bass as bass
import concourse.bacc as bacc
import concourse.tile as tile
import concourse.mybir as mybir
from concourse.bass import ts, ds
from concourse.bass_utils import run_bass_kernel_spmd

B, T, D, H, HD = 2, 2048, 1024, 16, 64
HPC = 4                # heads per core
E = HPC * HD           # 256 per-core channels
WP = 512               # projection chunk width (t)
WA = 256               # attention chunk width (q)
NPC = T // WP          # 4
NAC = T // WA          # 8
NKT = T // 128         # 16 k-tiles
DQ = D // 128          # 8 contraction subtiles
NEG = -10000.0
FP32 = mybir.dt.float32
BF16 = mybir.dt.bfloat16
SCALE = 1.0 / float(np.sqrt(HD))
NTT = T // 128         # 16 t-tiles for the output projection


def build_program(level=99):
    nc = bacc.Bacc("TRN2", target_bir_lowering=False, debug=False)
    xT_in = nc.declare_dram_parameter("xT_b", [D, T], BF16, isOutput=False)
    wqT = nc.declare_dram_parameter("wqT", [D, E], BF16, isOutput=False)
    wkT = nc.declare_dram_parameter("wkT", [D, E], BF16, isOutput=False)
    wvT = nc.declare_dram_parameter("wvT", [D, E], BF16, isOutput=False)
    wpT = nc.declare_dram_parameter("wpT", [E, D], BF16, isOutput=False)
    cosT = nc.declare_dram_parameter("cosT", [128, T], BF16, isOutput=False)
    sinT = nc.declare_dram_parameter("sinT", [128, T], BF16, isOutput=False)
    # dmask[:, idx, h, :] = causal 0/1 mask for the two diagonal k-tile
    # positions (idx 0: k-tile aligned with chunk start, idx 1: +128),
    # replicated over the 4 head slots.
    dmask = nc.declare_dram_parameter("dmask", [128, 2 * HPC * WA], BF16, isOutput=False)
    outp = nc.declare_dram_parameter("outp", [T, D], BF16, isOutput=True)

    with tile.TileContext(nc) as tc, ExitStack() as ctx:
        consts = ctx.enter_context(tc.tile_pool(name="consts", bufs=1))
        ropet = ctx.enter_context(tc.tile_pool(name="ropet", bufs=2))
        probs_p = ctx.enter_context(
            tc.tile_pool(name="probs", bufs=int(os.environ.get("K_PRBUFS", "2")))
        )
        asb_p = ctx.enter_context(tc.tile_pool(name="asb", bufs=2))
        den_p = ctx.enter_context(tc.tile_pool(name="den", bufs=2))
        ostage = ctx.enter_context(tc.tile_pool(name="ostage", bufs=2))
        warm_p = ctx.enter_context(tc.tile_pool(name="warm", bufs=1))
        ps_sc = ctx.enter_context(
            tc.tile_pool(
                name="ps_sc", bufs=int(os.environ.get("K_SCBUFS", "2")), space="PSUM"
            )
        )
        ps_acc = ctx.enter_context(tc.tile_pool(name="ps_acc", bufs=1, space="PSUM"))
        ps_io = ctx.enter_context(tc.tile_pool(name="ps_io", bufs=1, space="PSUM"))

        # ---- exp table prefetch: 1-element dummy activation at t~0 ----
        warm = warm_p.tile([1, 8], FP32, tag="warm")
        nc.vector.memset(warm[:, 0:4], 0.0)
        nc.scalar.activation(
            warm[:, 4:8], warm[:, 0:4], mybir.ActivationFunctionType.Exp, scale=1.0
        )
        # ---- PE HAM warm-up: dummy matmuls during the input-DMA wait ----
        # The PE clock-gate (HAM) needs ~3.4us of sustained matmul activity
        # to un-throttle from 1.2 to 2.4 GHz.  The input DMAs take ~11us,
        # so without this the whole prologue runs at half clock.  Dummy
        # N=128 matmuls on a memset tile keep the PE busy until real data
        # lands; they are ahead of the real work in the PE FIFO and finish
        # just before it becomes ready.
        zdum = warm_p.tile([128, 128], BF16, tag="zdum")
        nc.vector.memset(zdum[:], 0.0)
        # ~26 dummies run cold before HAM un-throttles (~3.4us); a few more
        # keep it warm until wq lands (~10.5us).  More than that delays the
        # first real LDWEIGHTS behind the dummy stream.
        n_warm = int(os.environ.get("K_WARM_MM", "40"))
        if n_warm:
            pwarm = ps_sc.tile([128, 128], FP32, tag="sc", name="pwarm")
            for _ in range(n_warm):
                nc.tensor.matmul(
                    pwarm[:],
                    lhsT=zdum[:],
                    rhs=zdum[:],
                    start=True,
                    stop=True,
                    skip_group_check=True,
                )

        # ---- constants / weights / x to SBUF, priority-ordered ----
        # Queue assignment (3 parallel DMA paths): sync=HW, scalar=HW,
        # gpsimd=SW.  Critical prefix: wq; xT j=0; wk; cos/sin j=0.
        xT_sb = consts.tile([128, DQ, T], BF16, tag="xT")
        xT_r = xT_in.rearrange("(o p) m -> p o m", p=128)
        wq_sb = consts.tile([128, DQ, E], BF16, tag="wq")
        wq_r = wqT.rearrange("(o p) m -> p o m", p=128)
        wk_sb = consts.tile([128, DQ, E], BF16, tag="wk")
        wk_r = wkT.rearrange("(o p) m -> p o m", p=128)
        wv_sb = consts.tile([128, DQ, E], BF16, tag="wv")
        wv_r = wvT.rearrange("(o p) m -> p o m", p=128)
        wp_sb = consts.tile([128, 2, D], BF16, tag="wp")
        cos_sb = consts.tile([128, T], BF16, tag="cos")
        sin_sb = consts.tile([128, T], BF16, tag="sin")
        dm_sb = consts.tile([128, 2, HPC, WA], BF16, tag="dmask")

        # wave 0: the prologue's critical inputs, spread across all three
        # DMA queues: wq (sync), xT j0 (all queues), cos/sin j0 (scalar).
        nc.sync.dma_start(wq_sb[:, 0:4, :], wq_r[:, 0:4, :])
        nc.sync.dma_start(wq_sb[:, 4:8, :], wq_r[:, 4:8, :])
        nc.scalar.dma_start(cos_sb[:, ts(0, WP)], cosT[:, ts(0, WP)])
        nc.scalar.dma_start(sin_sb[:, ts(0, WP)], sinT[:, ts(0, WP)])
        for dq in range(3):
            nc.scalar.dma_start(xT_sb[:, dq, ts(0, WP)], xT_r[:, dq, ts(0, WP)])
        for dq in range(3, 6):
            nc.gpsimd.dma_start(xT_sb[:, dq, ts(0, WP)], xT_r[:, dq, ts(0, WP)])
        for dq in range(6, 8):
            nc.sync.dma_start(xT_sb[:, dq, ts(0, WP)], xT_r[:, dq, ts(0, WP)])
        # wave 1: wk, wv, and xT j1 balanced across the queues so the j=1
        # projection quantum (drained during chunk 1) never head-of-line
        # blocks the PE on a straggling slice.
        nc.sync.dma_start(wk_sb[:, 0:4, :], wk_r[:, 0:4, :])
        nc.scalar.dma_start(wk_sb[:, 4:8, :], wk_r[:, 4:8, :])
        nc.gpsimd.dma_start(wv_sb[:, 0:4, :], wv_r[:, 0:4, :])
        nc.gpsimd.dma_start(wv_sb[:, 4:8, :], wv_r[:, 4:8, :])
        for dq in range(3):
            nc.scalar.dma_start(xT_sb[:, dq, ts(1, WP)], xT_r[:, dq, ts(1, WP)])
        for dq in range(3, 6):
            nc.gpsimd.dma_start(xT_sb[:, dq, ts(1, WP)], xT_r[:, dq, ts(1, WP)])
        for dq in range(6, 8):
            nc.sync.dma_start(xT_sb[:, dq, ts(1, WP)], xT_r[:, dq, ts(1, WP)])
        nc.scalar.dma_start(dm_sb[:], dmask[:])
        nc.scalar.dma_start(cos_sb[:, ts(1, WP)], cosT[:, ts(1, WP)])
        nc.scalar.dma_start(sin_sb[:, ts(1, WP)], sinT[:, ts(1, WP)])
        # v_aug[:, t, h, :]: even h = [v | ones], odd h = [ones | v]; fill
        # everything with ones, the v copies overwrite their halves.
        # memset on vector: the DVE is idle until the first projection psum
        # lands (~16us), so this is free there, and it keeps the gpsimd
        # engine clear for SW-DGE descriptor generation.
        v_aug = consts.tile([128, NKT, HPC, 128], BF16, tag="vaug")
        nc.vector.memset(v_aug[:], 1.0)

        # wave 2: wp (gpsimd), xT j2 (sync/gpsimd), cos/sin j2-3 (scalar)
        nc.gpsimd.dma_start(wp_sb[:], wpT.rearrange("(o p) m -> p o m", p=128))
        for dq in range(0, DQ, 2):
            nc.sync.dma_start(xT_sb[:, dq, ts(2, WP)], xT_r[:, dq, ts(2, WP)])
            nc.gpsimd.dma_start(
                xT_sb[:, dq + 1, ts(2, WP)], xT_r[:, dq + 1, ts(2, WP)]
            )
        for j in (2, 3):
            nc.scalar.dma_start(cos_sb[:, ts(j, WP)], cosT[:, ts(j, WP)])
            nc.scalar.dma_start(sin_sb[:, ts(j, WP)], sinT[:, ts(j, WP)])
        for dq in range(0, DQ, 2):
            nc.sync.dma_start(xT_sb[:, dq, ts(3, WP)], xT_r[:, dq, ts(3, WP)])
            nc.gpsimd.dma_start(
                xT_sb[:, dq + 1, ts(3, WP)], xT_r[:, dq + 1, ts(3, WP)]
            )

        # q/k pair tiles [128, T]: head 2p on partitions 0:64, head 2p+1 on
        # 64:128.  The scores matmuls for a pair run as two concurrent
        # 64-row PE tiles (tile_position (0,0) / (64,0)) — both halves of
        # the systolic array stream at once, halving scores wall time.
        q_nat = [
            consts.tile([128, T], BF16, tag=f"qnat{p}", name=f"qnat{p}")
            for p in range(2)
        ]
        k_nat = [
            consts.tile([128, T], BF16, tag=f"knat{p}", name=f"knat{p}")
            for p in range(2)
        ]
        # head h -> sct/pr column slot: pair members go to different psum
        # BANKS (h0/h2 in bank 0 = slots 0,1; h1/h3 in bank 1 = slots 2,3)
        # so the two concurrent streams never write the same bank.
        SLOT = [0, 2, 1, 3]
        attn_nrm = [
            consts.tile([128, T], BF16, tag=f"anrm{p}", name=f"anrm{p}")
            for p in range(2)
        ]

        # ---- work quanta (proj / outproj), drained between attn iters ----
        def emit_qk(j, w_sb, nat, dq_order=None, trickle=False):
            pqk = ps_io.tile([128, 2, WP], FP32, tag="io", name="pqk")
            dqs = dq_order if dq_order is not None else range(DQ)
            # dq-major so each matmul only needs its own xT slice (slices
            # arrive one at a time during the prologue); the interleaved
            # bank accumulation groups are fine on HW.
            for n, dq in enumerate(dqs):
                if trickle and n % 2 == 0:
                    # prologue matmuls stall on arriving xT slices; dummy
                    # LDWEIGHTS between them keep the HAM clock-gate warm
                    nc.tensor.ldweights(weights=zdum[:])
                    nc.tensor.ldweights(weights=zdum[:])
                for half in range(2):
                    nc.tensor.matmul(
                        pqk[:, half, :],
                        lhsT=w_sb[:, dq, ds(128 * half, 128)],
                        rhs=xT_sb[:, dq, ts(j, WP)],
                        start=(n == 0),
                        stop=(n == DQ - 1),
                        skip_group_check=True,
                    )
            lo, hi = pqk[:, 0, :], pqk[:, 1, :]
            cs, sn = cos_sb[:, ts(j, WP)], sin_sb[:, ts(j, WP)]
            st = ropet.tile([128, 2, WP], BF16, tag="st", name="st")
            # bf16 intermediates: the final add/sub then runs all-16-bit
            # operands at 2x DVE rate
            ta = ropet.tile([128, 2, WP], BF16, tag="ta", name="ta")
            tb = ropet.tile([128, 2, WP], BF16, tag="tb", name="tb")
            nc.vector.tensor_mul(ta[:, 0, :], lo, cs)
            nc.vector.tensor_mul(ta[:, 1, :], hi, sn)
            nc.vector.tensor_sub(st[:, 0, :], ta[:, 0, :], ta[:, 1, :])
            nc.vector.tensor_mul(tb[:, 0, :], hi, cs)
            nc.vector.tensor_mul(tb[:, 1, :], lo, sn)
            nc.vector.tensor_add(st[:, 1, :], tb[:, 0, :], tb[:, 1, :])
            # one combined DMA per head: rows land interleaved
            # (lo0,hi0,lo1,hi1,...) — scores are invariant to a row
            # permutation applied consistently to q and k.  Head h goes to
            # pair tile h//2, partition half h%2.
            for h in range(HPC):
                nc.sync.dma_start(
                    nat[h // 2][ds(64 * (h % 2), 64), ts(j, WP)],
                    st[ds(32 * h, 32), :, :],
                )

        def emit_v(j, half_pair):
            pv = ps_io.tile([128, 2, E], FP32, tag="io", name="pv")
            for tt in range(2):
                g = 4 * j + 2 * half_pair + tt
                for dq in range(DQ):
                    nc.tensor.matmul(
                        pv[:, tt, :],
                        lhsT=xT_sb[:, dq, ts(g, 128)],
                        rhs=wv_sb[:, dq, :],
                        start=(dq == 0),
                        stop=(dq == DQ - 1),
                    )
            for tt in range(2):
                g = 4 * j + 2 * half_pair + tt
                for h in range(HPC):
                    voff = 0 if h % 2 == 0 else 64
                    nc.vector.tensor_copy(
                        v_aug[:, g, h, ds(voff, 64)], pv[:, tt, ds(64 * h, 64)]
                    )

        def emit_po(g, pool=None, tag="io", tail=False, hwq=False):
            po = (pool or ps_io).tile([128, D], FP32, tag=tag, name="po")
            if tail:
                # tail tiles pipeline per 512-col half: cast + store of the
                # first half run while the second half's matmuls stream.
                # Stores ride the HW DGE queues (sync/scalar); by the tail
                # the exp stream is finished so scalar is free.
                for dh in range(2):
                    for p in range(2):
                        nc.tensor.matmul(
                            po[:, ds(512 * dh, 512)],
                            lhsT=attn_nrm[p][:, ts(g, 128)],
                            rhs=wp_sb[:, p, ds(512 * dh, 512)],
                            start=(p == 0),
                            stop=(p == 1),
                        )
                    ost = ostage.tile([128, 512], BF16, tag="ost", name="ost")
                    if (g + dh) % 2 == 1:
                        nc.scalar.copy(ost[:], po[:, ds(512 * dh, 512)])
                    else:
                        nc.vector.tensor_copy(ost[:], po[:, ds(512 * dh, 512)])
                    deng = nc.scalar if g % 2 == 1 else nc.sync
                    deng.dma_start(
                        outp[ts(g, 128), ds(512 * dh, 512)], ost[:]
                    )
                return
            for dh in range(2):
                for p in range(2):
                    nc.tensor.matmul(
                        po[:, ds(512 * dh, 512)],
                        lhsT=attn_nrm[p][:, ts(g, 128)],
                        rhs=wp_sb[:, p, ds(512 * dh, 512)],
                        start=(p == 0),
                        stop=(p == 1),
                    )
            ost = ostage.tile([128, D], BF16, tag="ost", name="ost")
            nc.vector.tensor_copy(ost[:], po[:])
            if hwq:
                # near-tail stores ride the HW sync queue; the gpsimd SW
                # queue drains too slowly to sit on the exit path
                nc.sync.dma_start(outp[ts(g, 128), :], ost[:])
            elif g % 2 == 0:
                nc.gpsimd.dma_start(outp[ts(g, 128), :], ost[:])
            else:
                nc.sync.dma_start(outp[ts(g, 128), :], ost[:])

        pending = []
        gap = [0]
        drain_every = [1]

        def drain_one():
            if pending and gap[0] >= drain_every[0]:
                pending.pop(0)()
                gap[0] = 0



        # prologue: projection chunk 0 (serial; nothing to overlap with yet)
        # dq consumption ordered by DMA arrival: gpsimd slices (3,4,5) and
        # scalar slices (0,1,2) land before the sync ones (6,7 behind wq).
        if level >= 1 and not os.environ.get("K_NOPRO"):
            arrival = [3, 4, 5, 0, 1, 2, 6, 7]
            emit_qk(0, wq_sb, q_nat, dq_order=arrival, trickle=True)
            emit_qk(0, wk_sb, k_nat, dq_order=arrival, trickle=True)
            emit_v(0, 0)
            emit_v(0, 1)

        # chunk processing order: chunk 0 second — its inputs are already
        # resident, so it fills the PE while the xT j=1 slices land; chunk 6
        # last so chunk 7's outproj tiles drain during it, leaving only
        # g12,13 for the tail.
        order = [1, 0, 2, 3, 4, 5, 7, 6]
        n_pos = {0: 0, 1: 0, 2: 1, 3: 4, 4: NAC}.get(level, NAC)
        if os.environ.get("K_NCHUNKS"):
            n_pos = int(os.environ["K_NCHUNKS"])
        # ---- attention chunks, with quanta interleaved ----
        for pos in range(n_pos):
            a = order[pos]
            # draining faster than every 2 iters front-loads quanta whose
            # input DMAs haven't landed and head-of-line blocks the PE FIFO
            drain_every[0] = 2
            if level >= 3 and not os.environ.get("K_NOQUANTA"):
                if pos in (0, 2):
                    j = 1 if pos == 0 else 2
                    pending.append(lambda j=j: emit_qk(j, wq_sb, q_nat))
                    pending.append(lambda j=j: emit_qk(j, wk_sb, k_nat))
                    pending.append(lambda j=j: emit_v(j, 0))
                    pending.append(lambda j=j: emit_v(j, 1))
                elif pos == 3:
                    pending.append(lambda: emit_qk(3, wq_sb, q_nat))
                    pending.append(lambda: emit_qk(3, wk_sb, k_nat))
                elif pos == 6:
                    # v j=3 deferred into chunk 7's ACT-paced slack: its
                    # tiles are only consumed from iteration 13 onward,
                    # and pos 3-5 are already PE-oversubscribed.
                    pending.append(lambda: emit_v(3, 0))
                    pending.append(lambda: emit_v(3, 1))
            if level >= 5:
                po_sched = {
                    3: (0, 1),           # chunk 0 (processed at pos 1)
                    4: (2, 3, 4),        # chunks 1,2
                    5: (5, 6, 7, 8, 9),  # chunks 2,3,4
                    6: (10, 11),         # chunk 5
                    7: (14, 15),         # chunk 7 (processed at pos 6)
                }
                for g in po_sched.get(pos, ()):
                    pending.append(lambda g=g: emit_po(g))

            nk = 2 * a + 2
            asum = ps_acc.tile([128, HPC, WA], FP32, tag="acc", name="asum")

            def S(i, a=a):
                sct = ps_sc.tile([128, HPC, WA], FP32, tag="sc", name="sct")
                # packed pairs: heads 2p / 2p+1 stream concurrently through
                # PE row-groups 0:64 / 64:128 into different psum banks.
                # start=True only on each bank's first writer (h0 -> bank0,
                # h1 -> bank1); the second writer overwrites fresh since the
                # bank's has_written bits were cleared.
                for h in range(HPC):
                    p, half = h // 2, h % 2
                    nc.tensor.matmul(
                        sct[:, SLOT[h], :],
                        lhsT=k_nat[p][ds(64 * half, 64), ts(i, 128)],
                        rhs=q_nat[p][ds(64 * half, 64), ts(a, WA)],
                        start=(h < 2),
                        stop=True,
                        tile_position=(64 * half, 0),
                        skip_group_check=True,
                    )
                return sct

            def EPV(i, sct, nk=nk, asum=asum, a=a):
                pr = probs_p.tile([128, HPC, WA], BF16, tag="pr", name="pr")
                nc.scalar.activation(
                    pr[:], sct[:], mybir.ActivationFunctionType.Exp, scale=SCALE
                )
                if i >= 2 * a:
                    # diagonal k-tile: zero the masked entries on the DVE
                    # (exp(s)*mask == softmax numerator with -inf masking)
                    nc.vector.tensor_mul(pr[:], pr[:], dm_sb[:, i - 2 * a, :, :])
                for h in range(HPC):
                    # i==0, even h: start=True clears the whole bank's
                    # has_written bits; the odd head's first matmul then
                    # overwrites (its bits are clear) — no zero-init needed.
                    nc.tensor.matmul(
                        asum[:, h, :],
                        lhsT=v_aug[:, i, h, :],
                        rhs=pr[:, SLOT[h], :],
                        start=(i == 0 and h % 2 == 0),
                        stop=(i == nk - 1),
                        skip_group_check=True,
                    )

            prev = None
            for i in range(nk):
                sct = S(i)
                if prev is not None:
                    EPV(prev[0], prev[1])
                prev = (i, sct)
                gap[0] += 1
                drain_one()
                if pos < 3:
                    # HAM keep-warm trickle: dummy LDWEIGHTS during the
                    # early dependency gaps keep the PE activity monitor
                    # from re-throttling the clock to 1.2 GHz (two are
                    # needed — one alone measurably fails to hold K=8/8).
                    # Harmless: every real matmul reloads its own weights.
                    # pos 3-4 are quanta-dense: trickles there cost more
                    # inline time than the cold they prevent.
                    nc.tensor.ldweights(weights=zdum[:])
                    nc.tensor.ldweights(weights=zdum[:])
            EPV(prev[0], prev[1])
            # chunk-boundary trickle: the epilogue hand-off can idle the PE
            # past the HAM window; two dummy LDWEIGHTS hold the clock warm
            nc.tensor.ldweights(weights=zdum[:])
            nc.tensor.ldweights(weights=zdum[:])

            if os.environ.get("K_NOEPI"):
                continue
            if pos == n_pos - 1 and not os.environ.get("K_NOFASTEPI"):
                # tail fast path: stage only the DENOMINATOR halves to SBUF
                # (half the copy bytes), swap, reciprocal, and multiply
                # reading the numerators straight from psum — ~2us shorter
                # chain before the last outproj tiles can start.
                sd = asb_p.tile([128, 2, WA], FP32, tag="asb", name="sd")
                den = den_p.tile([128, 2, WA], FP32, tag="den", name="den")
                rc = den_p.tile([128, 2, WA], FP32, tag="rc", name="rc")
                nc.vector.tensor_copy(sd[ds(64, 64), :, :], asum[ds(64, 64), 0:4:2, :])
                nc.scalar.copy(sd[ds(0, 64), :, :], asum[ds(0, 64), 1:4:2, :])
                nc.sync.dma_start(den[ds(0, 64), :, :], sd[ds(64, 64), :, :])
                nc.sync.dma_start(den[ds(64, 64), :, :], sd[ds(0, 64), :, :])
                nc.vector.reciprocal_approx_fast(rc[:], den[:])
                for p in range(2):
                    nc.vector.tensor_mul(
                        attn_nrm[p][ds(0, 64), ts(a, WA)],
                        asum[ds(0, 64), 2 * p, :],
                        rc[ds(0, 64), p, :],
                    )
                    nc.vector.tensor_mul(
                        attn_nrm[p][ds(64, 64), ts(a, WA)],
                        asum[ds(64, 64), 2 * p + 1, :],
                        rc[ds(64, 64), p, :],
                    )
                continue
            # epilogue: drain asum per bank-pair (vector), den swap per pair
            # right behind its copy, reciprocal + normalize per pair — the
            # pair-0 chain completes ~1us earlier than a monolithic drain.
            asb = asb_p.tile([128, HPC, WA], FP32, tag="asb", name="asb")
            den = den_p.tile([128, 2, WA], FP32, tag="den", name="den")
            rc = den_p.tile([128, 2, WA], FP32, tag="rc", name="rc")
            # in the fill phase (and the tail, where the exp stream has
            # ended) the DVE is the bottleneck — scalar takes the asum
            # drains there so the reciprocal+normalize chain starts sooner
            drain_scalar = pos < 3 or pos == n_pos - 1
            if drain_scalar:
                nc.scalar.copy(asb[:, 0:2, :], asum[:, 0:2, :])
            else:
                nc.vector.tensor_copy(asb[:, 0:2, :], asum[:, 0:2, :])
            nc.sync.dma_start(den[ds(0, 64), 0, :], asb[ds(64, 64), 0, :])
            nc.sync.dma_start(den[ds(64, 64), 0, :], asb[ds(0, 64), 1, :])
            if drain_scalar:
                nc.scalar.copy(asb[:, 2:4, :], asum[:, 2:4, :])
            else:
                nc.vector.tensor_copy(asb[:, 2:4, :], asum[:, 2:4, :])
            nc.sync.dma_start(den[ds(0, 64), 1, :], asb[ds(64, 64), 2, :])
            nc.sync.dma_start(den[ds(64, 64), 1, :], asb[ds(0, 64), 3, :])
            for p in range(2):
                nc.vector.reciprocal_approx_fast(rc[:, p, :], den[:, p, :])
                nc.vector.tensor_mul(
                    attn_nrm[p][ds(0, 64), ts(a, WA)],
                    asb[ds(0, 64), 2 * p, :],
                    rc[ds(0, 64), p, :],
                )
                nc.vector.tensor_mul(
                    attn_nrm[p][ds(64, 64), ts(a, WA)],
                    asb[ds(64, 64), 2 * p + 1, :],
                    rc[ds(64, 64), p, :],
                )

        # tail: whatever quanta remain + chunk 6's output tiles
        for f in pending:
            f()
        if level >= 5:
            # tail tiles go in the now-idle scores pool so they run in
            # parallel instead of serializing on the single-buffer io ring
            for g in (12, 13):
                emit_po(g, pool=ps_sc, tag="sc", tail=True)

    nc.compile()
    return nc


def make_consts(cos, sin):
    cosT = np.ascontiguousarray(
        np.tile(np.asarray(cos[0], dtype=np.float32).T[:32], (4, 1))
    ).astype(ml_dtypes.bfloat16)
    sinT = np.ascontiguousarray(
        np.tile(np.asarray(sin[0], dtype=np.float32).T[:32], (4, 1))
    ).astype(ml_dtypes.bfloat16)
    # dmask[kp, idx*HPC*WA + h*WA + qc] = 1 if (128*idx + kp) <= qc else 0
    kp = np.arange(128)[:, None]
    qc = np.arange(WA)[None, :]
    dm = np.stack(
        [
            np.repeat(((128 * idx + kp) <= qc)[:, None, :], HPC, axis=1)
            for idx in range(2)
        ],
        axis=1,
    )  # [128, 2, HPC, WA]
    dmask = np.ascontiguousarray(
        dm.reshape(128, 2 * HPC * WA).astype(ml_dtypes.bfloat16)
    )
    return dict(cosT=cosT, sinT=sinT, dmask=dmask)


def host_prep(core, xT_by_batch, Wq, Wk, Wv, Wp, consts):
    b, hp = core // 4, core % 4
    h0 = hp * HPC
    rows = slice(HD * h0, HD * h0 + E)
    Wq_s = np.asarray(Wq[rows]).reshape(HPC, HD, D)
    Wk_s = np.asarray(Wk[rows]).reshape(HPC, HD, D)
    wqT = np.ascontiguousarray(
        np.concatenate(
            [Wq_s[:, :32].reshape(128, D), Wq_s[:, 32:].reshape(128, D)], 0
        ).T.astype(ml_dtypes.bfloat16)
    )
    wkT = np.ascontiguousarray(
        np.concatenate(
            [Wk_s[:, :32].reshape(128, D), Wk_s[:, 32:].reshape(128, D)], 0
        ).T.astype(ml_dtypes.bfloat16)
    )
    wvT = np.ascontiguousarray(np.asarray(Wv[rows]).T.astype(ml_dtypes.bfloat16))
    wpT = np.ascontiguousarray(np.asarray(Wp[:, rows]).T.astype(ml_dtypes.bfloat16))
    return dict(
        xT_b=xT_by_batch[b],
        wqT=wqT,
        wkT=wkT,
        wvT=wvT,
        wpT=wpT,
        **consts,
    )


_NC_CACHE = None


def _get_nc():
    global _NC_CACHE
    if _NC_CACHE is None:
        _NC_CACHE = build_program()
    return _NC_CACHE


def kernel(x, cos, sin, Wq, Wk, Wv, Wp, _want_trace=False):
    x, cos, sin = np.asarray(x), np.asarray(cos), np.asarray(sin)
    Wq, Wk, Wv, Wp = (np.asarray(a) for a in (Wq, Wk, Wv, Wp))
    nc = _get_nc()
    consts = make_consts(cos, sin)
    xT_by_batch = [
        np.ascontiguousarray(x[b].T.astype(ml_dtypes.bfloat16)) for b in range(B)
    ]
    in_maps = [
        host_prep(core, xT_by_batch, Wq, Wk, Wv, Wp, consts) for core in range(8)
    ]
    res = run_bass_kernel_spmd(nc, in_maps, list(range(8)), trace=_want_trace)
    out = np.zeros((B, T, D), dtype=np.float32)
    for core in range(8):
        out[core // 4] += np.asarray(res.results[core]["outp"], dtype=np.float32)
    if _want_trace:
        kernel.last_exec_time_ns = res.exec_time_ns
        kernel.last_profile = res.profile_json
    return out


# revision 66
# speedup vs baseline: 1.0103x; 1.0103x over previous
"""Trainium2 Bass kernel for nn_MultiHeadAttention (B=2,T=2048,D=1024,H=16,HD=64).

Sharding: 8 cores = 2 batches x 4 heads/core (tensor parallel over heads).
Each core computes q,k,v projections for its 4 heads, RoPE, causal
flash-attention, and a partial output projection (its heads' slice of Wp);
the host sums the 4 partials per batch.

v3 design (on top of the fully-pipelined v2), measured 223us -> ~194us:
  - Packed scores matmuls: q/k stored as head-PAIR tiles [128, T]; each
    pair's two 64-row scores matmuls run CONCURRENTLY in the two PE
    array row-group halves (tile_position (0,0)/(64,0)) into different
    psum banks — halves scores wall time.
  - Causal diagonal masking moved off the PE: exp output is multiplied
    by a precomputed 0/1 tile on the DVE (kills 64 mask matmuls + the
    u/l mask weights).
  - PE HAM warm-up: dummy matmuls during the initial DMA wait plus a
    2x dummy-LDWEIGHTS trickle per early iteration hold the PE clock at
    2.4 GHz through the fill phase.
  - Startup: input DMAs split into column chunks and priority-ordered
    across the three DMA queues (sync-HW, scalar-HW, gpsimd-SW) so the
    first projection matmul starts at ~10.5us instead of ~19us.
  - exp activation table prefetched with a dummy 1-element exp at t=0.
  - Chunk order [1,0,2,3,4,5,7,6]: chunk 0 (inputs already resident)
    fills the xT j=1 DMA window; small tail chain hangs off chunk 6
    with a fast-path epilogue (reciprocal+normalize read psum direct).
  - Zero-init matmuls for the PV accumulator removed: the first PV
    matmul of each psum bank uses start=True (clears the whole bank's
    has_written bits; the co-banked head's first matmul then overwrites
    since its bits are clear).
  - q/k RoPE'd tiles stored with lo/hi rows interleaved (one combined
    DMA per head instead of two): scores are invariant to any row
    permutation applied consistently to q and k.
  - Output stored bf16 (host accumulates partials in fp32); tail
    stores pipelined per 512-col half on the HW DGE queues.
"""

import os
import sys

sys.path.insert(0, "/opt/trn_rl_repo")

from contextlib import ExitStack

import numpy as np
import ml_dtypes

import concourse.bass as bass
import concourse.bacc as bacc
import concourse.tile as tile
import concourse.mybir as mybir
from concourse.bass import ts, ds
from concourse.bass_utils import run_bass_kernel_spmd

B, T, D, H, HD = 2, 2048, 1024, 16, 64
HPC = 4                # heads per core
E = HPC * HD           # 256 per-core channels
WP = 512               # projection chunk width (t)
WA = 256               # attention chunk width (q)
NPC = T // WP          # 4
NAC = T // WA          # 8
NKT = T // 128         # 16 k-tiles
DQ = D // 128          # 8 contraction subtiles
NEG = -10000.0
FP32 = mybir.dt.float32
BF16 = mybir.dt.bfloat16
SCALE = 1.0 / float(np.sqrt(HD))
NTT = T // 128         # 16 t-tiles for the output projection


def build_program(level=99):
    nc = bacc.Bacc("TRN2", target_bir_lowering=False, debug=False)
    xT_in = nc.declare_dram_parameter("xT_b", [D, T], BF16, isOutput=False)
    wqT = nc.declare_dram_parameter("wqT", [D, E], BF16, isOutput=False)
    wkT = nc.declare_dram_parameter("wkT", [D, E], BF16, isOutput=False)
    wvT = nc.declare_dram_parameter("wvT", [D, E], BF16, isOutput=False)
    wpT = nc.declare_dram_parameter("wpT", [E, D], BF16, isOutput=False)
    cosT = nc.declare_dram_parameter("cosT", [128, T], BF16, isOutput=False)
    sinT = nc.declare_dram_parameter("sinT", [128, T], BF16, isOutput=False)
    # dmask[:, idx, h, :] = causal 0/1 mask for the two diagonal k-tile
    # positions (idx 0: k-tile aligned with chunk start, idx 1: +128),
    # replicated over the 4 head slots.
    dmask = nc.declare_dram_parameter("dmask", [128, 2 * HPC * WA], BF16, isOutput=False)
    outp = nc.declare_dram_parameter("outp", [T, D], BF16, isOutput=True)

    with tile.TileContext(nc) as tc, ExitStack() as ctx:
        consts = ctx.enter_context(tc.tile_pool(name="consts", bufs=1))
        ropet = ctx.enter_context(tc.tile_pool(name="ropet", bufs=2))
        probs_p = ctx.enter_context(
            tc.tile_pool(name="probs", bufs=int(os.environ.get("K_PRBUFS", "2")))
        )
        asb_p = ctx.enter_context(tc.tile_pool(name="asb", bufs=2))
        den_p = ctx.enter_context(tc.tile_pool(name="den", bufs=2))
        ostage = ctx.enter_context(tc.tile_pool(name="ostage", bufs=2))
        warm_p = ctx.enter_context(tc.tile_pool(name="warm", bufs=1))
        ps_sc = ctx.enter_context(
            tc.tile_pool(
                name="ps_sc", bufs=int(os.environ.get("K_SCBUFS", "2")), space="PSUM"
            )
        )
        ps_acc = ctx.enter_context(tc.tile_pool(name="ps_acc", bufs=1, space="PSUM"))
        ps_io = ctx.enter_context(tc.tile_pool(name="ps_io", bufs=1, space="PSUM"))

        # ---- exp table prefetch: 1-element dummy activation at t~0 ----
        warm = warm_p.tile([1, 8], FP32, tag="warm")
        nc.vector.memset(warm[:, 0:4], 0.0)
        nc.scalar.activation(
            warm[:, 4:8], warm[:, 0:4], mybir.ActivationFunctionType.Exp, scale=1.0
        )
        # ---- PE HAM warm-up: dummy matmuls during the input-DMA wait ----
        # The PE clock-gate (HAM) needs ~3.4us of sustained matmul activity
        # to un-throttle from 1.2 to 2.4 GHz.  The input DMAs take ~11us,
        # so without this the whole prologue runs at half clock.  Dummy
        # N=128 matmuls on a memset tile keep the PE busy until real data
        # lands; they are ahead of the real work in the PE FIFO and finish
        # just before it becomes ready.
        zdum = warm_p.tile([128, 128], BF16, tag="zdum")
        nc.vector.memset(zdum[:], 0.0)
        # ~26 dummies run cold before HAM un-throttles (~3.4us); a few more
        # keep it warm until wq lands (~10.5us).  More than that delays the
        # first real LDWEIGHTS behind the dummy stream.
        n_warm = int(os.environ.get("K_WARM_MM", "40"))
        if n_warm:
            pwarm = ps_sc.tile([128, 128], FP32, tag="sc", name="pwarm")
            for _ in range(n_warm):
                nc.tensor.matmul(
                    pwarm[:],
                    lhsT=zdum[:],
                    rhs=zdum[:],
                    start=True,
                    stop=True,
                    skip_group_check=True,
                )

        # ---- constants / weights / x to SBUF, priority-ordered ----
        # Queue assignment (3 parallel DMA paths): sync=HW, scalar=HW,
        # gpsimd=SW.  Critical prefix: wq; xT j=0; wk; cos/sin j=0.
        xT_sb = consts.tile([128, DQ, T], BF16, tag="xT")
        xT_r = xT_in.rearrange("(o p) m -> p o m", p=128)
        wq_sb = consts.tile([128, DQ, E], BF16, tag="wq")
        wq_r = wqT.rearrange("(o p) m -> p o m", p=128)
        wk_sb = consts.tile([128, DQ, E], BF16, tag="wk")
        wk_r = wkT.rearrange("(o p) m -> p o m", p=128)
        wv_sb = consts.tile([128, DQ, E], BF16, tag="wv")
        wv_r = wvT.rearrange("(o p) m -> p o m", p=128)
        wp_sb = consts.tile([128, 2, D], BF16, tag="wp")
        cos_sb = consts.tile([128, T], BF16, tag="cos")
        sin_sb = consts.tile([128, T], BF16, tag="sin")
        dm_sb = consts.tile([128, 2, HPC, WA], BF16, tag="dmask")

        # wave 0: the prologue's critical inputs, spread across all three
        # DMA queues: wq (sync), xT j0 (all queues), cos/sin j0 (scalar).
        nc.sync.dma_start(wq_sb[:, 0:4, :], wq_r[:, 0:4, :])
        nc.sync.dma_start(wq_sb[:, 4:8, :], wq_r[:, 4:8, :])
        nc.scalar.dma_start(cos_sb[:, ts(0, WP)], cosT[:, ts(0, WP)])
        nc.scalar.dma_start(sin_sb[:, ts(0, WP)], sinT[:, ts(0, WP)])
        for dq in range(3):
            nc.scalar.dma_start(xT_sb[:, dq, ts(0, WP)], xT_r[:, dq, ts(0, WP)])
        for dq in range(3, 6):
            nc.gpsimd.dma_start(xT_sb[:, dq, ts(0, WP)], xT_r[:, dq, ts(0, WP)])
        for dq in range(6, 8):
            nc.sync.dma_start(xT_sb[:, dq, ts(0, WP)], xT_r[:, dq, ts(0, WP)])
        # wave 1: wk, wv, and xT j1 balanced across the queues so the j=1
        # projection quantum (drained during chunk 1) never head-of-line
        # blocks the PE on a straggling slice.
        nc.sync.dma_start(wk_sb[:, 0:4, :], wk_r[:, 0:4, :])
        nc.scalar.dma_start(wk_sb[:, 4:8, :], wk_r[:, 4:8, :])
        nc.gpsimd.dma_start(wv_sb[:, 0:4, :], wv_r[:, 0:4, :])
        nc.gpsimd.dma_start(wv_sb[:, 4:8, :], wv_r[:, 4:8, :])
        for dq in range(3):
            nc.scalar.dma_start(xT_sb[:, dq, ts(1, WP)], xT_r[:, dq, ts(1, WP)])
        for dq in range(3, 6):
            nc.gpsimd.dma_start(xT_sb[:, dq, ts(1, WP)], xT_r[:, dq, ts(1, WP)])
        for dq in range(6, 8):
            nc.sync.dma_start(xT_sb[:, dq, ts(1, WP)], xT_r[:, dq, ts(1, WP)])
        nc.scalar.dma_start(dm_sb[:], dmask[:])
        nc.scalar.dma_start(cos_sb[:, ts(1, WP)], cosT[:, ts(1, WP)])
        nc.scalar.dma_start(sin_sb[:, ts(1, WP)], sinT[:, ts(1, WP)])
        # v_aug[:, t, h, :]: even h = [v | ones], odd h = [ones | v]; fill
        # everything with ones, the v copies overwrite their halves.
        # memset on vector: the DVE is idle until the first projection psum
        # lands (~16us), so this is free there, and it keeps the gpsimd
        # engine clear for SW-DGE descriptor generation.
        v_aug = consts.tile([128, NKT, HPC, 128], BF16, tag="vaug")
        nc.vector.memset(v_aug[:], 1.0)

        # wave 2: wp (gpsimd), xT j2 (sync/gpsimd), cos/sin j2-3 (scalar)
        nc.gpsimd.dma_start(wp_sb[:], wpT.rearrange("(o p) m -> p o m", p=128))
        for dq in range(0, DQ, 2):
            nc.sync.dma_start(xT_sb[:, dq, ts(2, WP)], xT_r[:, dq, ts(2, WP)])
            nc.gpsimd.dma_start(
                xT_sb[:, dq + 1, ts(2, WP)], xT_r[:, dq + 1, ts(2, WP)]
            )
        for j in (2, 3):
            nc.scalar.dma_start(cos_sb[:, ts(j, WP)], cosT[:, ts(j, WP)])
            nc.scalar.dma_start(sin_sb[:, ts(j, WP)], sinT[:, ts(j, WP)])
        for dq in range(0, DQ, 2):
            nc.sync.dma_start(xT_sb[:, dq, ts(3, WP)], xT_r[:, dq, ts(3, WP)])
            nc.gpsimd.dma_start(
                xT_sb[:, dq + 1, ts(3, WP)], xT_r[:, dq + 1, ts(3, WP)]
            )

        # q/k pair tiles [128, T]: head 2p on partitions 0:64, head 2p+1 on
        # 64:128.  The scores matmuls for a pair run as two concurrent
        # 64-row PE tiles (tile_position (0,0) / (64,0)) — both halves of
        # the systolic array stream at once, halving scores wall time.
        q_nat = [
            consts.tile([128, T], BF16, tag=f"qnat{p}", name=f"qnat{p}")
            for p in range(2)
        ]
        k_nat = [
            consts.tile([128, T], BF16, tag=f"knat{p}", name=f"knat{p}")
            for p in range(2)
        ]
        # head h -> sct/pr column slot: pair members go to different psum
        # BANKS (h0/h2 in bank 0 = slots 0,1; h1/h3 in bank 1 = slots 2,3)
        # so the two concurrent streams never write the same bank.
        SLOT = [0, 2, 1, 3]
        attn_nrm = [
            consts.tile([128, T], BF16, tag=f"anrm{p}", name=f"anrm{p}")
            for p in range(2)
        ]

        # ---- work quanta (proj / outproj), drained between attn iters ----
        def emit_qk(j, w_sb, nat, dq_order=None, trickle=False):
            pqk = ps_io.tile([128, 2, WP], FP32, tag="io", name="pqk")
            dqs = dq_order if dq_order is not None else range(DQ)
            # dq-major so each matmul only needs its own xT slice (slices
            # arrive one at a time during the prologue); the interleaved
            # bank accumulation groups are fine on HW.
            for n, dq in enumerate(dqs):
                if trickle and n % 2 == 0:
                    # prologue matmuls stall on arriving xT slices; dummy
                    # LDWEIGHTS between them keep the HAM clock-gate warm
                    nc.tensor.ldweights(weights=zdum[:])
                    nc.tensor.ldweights(weights=zdum[:])
                for half in range(2):
                    nc.tensor.matmul(
                        pqk[:, half, :],
                        lhsT=w_sb[:, dq, ds(128 * half, 128)],
                        rhs=xT_sb[:, dq, ts(j, WP)],
                        start=(n == 0),
                        stop=(n == DQ - 1),
                        skip_group_check=True,
                    )
            lo, hi = pqk[:, 0, :], pqk[:, 1, :]
            cs, sn = cos_sb[:, ts(j, WP)], sin_sb[:, ts(j, WP)]
            st = ropet.tile([128, 2, WP], BF16, tag="st", name="st")
            # bf16 intermediates: the final add/sub then runs all-16-bit
            # operands at 2x DVE rate
            ta = ropet.tile([128, 2, WP], BF16, tag="ta", name="ta")
            tb = ropet.tile([128, 2, WP], BF16, tag="tb", name="tb")
            nc.vector.tensor_mul(ta[:, 0, :], lo, cs)
            nc.vector.tensor_mul(ta[:, 1, :], hi, sn)
            nc.vector.tensor_sub(st[:, 0, :], ta[:, 0, :], ta[:, 1, :])
            nc.vector.tensor_mul(tb[:, 0, :], hi, cs)
            nc.vector.tensor_mul(tb[:, 1, :], lo, sn)
            nc.vector.tensor_add(st[:, 1, :], tb[:, 0, :], tb[:, 1, :])
            # one combined DMA per head: rows land interleaved
            # (lo0,hi0,lo1,hi1,...) — scores are invariant to a row
            # permutation applied consistently to q and k.  Head h goes to
            # pair tile h//2, partition half h%2.
            for h in range(HPC):
                nc.sync.dma_start(
                    nat[h // 2][ds(64 * (h % 2), 64), ts(j, WP)],
                    st[ds(32 * h, 32), :, :],
                )

        def emit_v(j, half_pair):
            pv = ps_io.tile([128, 2, E], FP32, tag="io", name="pv")
            for tt in range(2):
                g = 4 * j + 2 * half_pair + tt
                for dq in range(DQ):
                    nc.tensor.matmul(
                        pv[:, tt, :],
                        lhsT=xT_sb[:, dq, ts(g, 128)],
                        rhs=wv_sb[:, dq, :],
                        start=(dq == 0),
                        stop=(dq == DQ - 1),
                    )
            for tt in range(2):
                g = 4 * j + 2 * half_pair + tt
                for h in range(HPC):
                    voff = 0 if h % 2 == 0 else 64
                    nc.vector.tensor_copy(
                        v_aug[:, g, h, ds(voff, 64)], pv[:, tt, ds(64 * h, 64)]
                    )

        def emit_po(g, pool=None, tag="io", tail=False, hwq=False):
            po = (pool or ps_io).tile([128, D], FP32, tag=tag, name="po")
            if tail:
                # tail tiles pipeline per 512-col half: cast + store of the
                # first half run while the second half's matmuls stream.
                # Stores ride the HW DGE queues (sync/scalar); by the tail
                # the exp stream is finished so scalar is free.
                for dh in range(2):
                    for p in range(2):
                        nc.tensor.matmul(
                            po[:, ds(512 * dh, 512)],
                            lhsT=attn_nrm[p][:, ts(g, 128)],
                            rhs=wp_sb[:, p, ds(512 * dh, 512)],
                            start=(p == 0),
                            stop=(p == 1),
                        )
                    ost = ostage.tile([128, 512], BF16, tag="ost", name="ost")
                    if (g + dh) % 2 == 1:
                        nc.scalar.copy(ost[:], po[:, ds(512 * dh, 512)])
                    else:
                        nc.vector.tensor_copy(ost[:], po[:, ds(512 * dh, 512)])
                    deng = nc.scalar if g % 2 == 1 else nc.sync
                    deng.dma_start(
                        outp[ts(g, 128), ds(512 * dh, 512)], ost[:]
                    )
                return
            for dh in range(2):
                for p in range(2):
                    nc.tensor.matmul(
                        po[:, ds(512 * dh, 512)],
                        lhsT=attn_nrm[p][:, ts(g, 128)],
                        rhs=wp_sb[:, p, ds(512 * dh, 512)],
                        start=(p == 0),
                        stop=(p == 1),
                    )
            ost = ostage.tile([128, D], BF16, tag="ost", name="ost")
            nc.vector.tensor_copy(ost[:], po[:])
            if hwq:
                # near-tail stores ride the HW sync queue; the gpsimd SW
                # queue drains too slowly to sit on the exit path
                nc.sync.dma_start(outp[ts(g, 128), :], ost[:])
            elif g % 2 == 0:
                nc.gpsimd.dma_start(outp[ts(g, 128), :], ost[:])
            else:
                nc.sync.dma_start(outp[ts(g, 128), :], ost[:])

        pending = []
        gap = [0]
        drain_every = [1]

        def drain_one():
            if pending and gap[0] >= drain_every[0]:
                pending.pop(0)()
                gap[0] = 0



        # prologue: projection chunk 0 (serial; nothing to overlap with yet)
        # dq consumption ordered by DMA arrival: gpsimd slices (3,4,5) and
        # scalar slices (0,1,2) land before the sync ones (6,7 behind wq).
        if level >= 1 and not os.environ.get("K_NOPRO"):
            arrival = [3, 4, 5, 0, 1, 2, 6, 7]
            emit_qk(0, wq_sb, q_nat, dq_order=arrival, trickle=True)
            emit_qk(0, wk_sb, k_nat, dq_order=arrival, trickle=True)
            emit_v(0, 0)
            emit_v(0, 1)

        # chunk processing order: chunk 0 second — its inputs are already
        # resident, so it fills the PE while the xT j=1 slices land; chunk 6
        # last so chunk 7's outproj tiles drain during it, leaving only
        # g12,13 for the tail.
        order = [1, 0, 2, 3, 4, 5, 7, 6]
        n_pos = {0: 0, 1: 0, 2: 1, 3: 4, 4: NAC}.get(level, NAC)
        if os.environ.get("K_NCHUNKS"):
            n_pos = int(os.environ["K_NCHUNKS"])
        # ---- attention chunks, with quanta interleaved ----
        for pos in range(n_pos):
            a = order[pos]
            # draining faster than every 2 iters front-loads quanta whose
            # input DMAs haven't landed and head-of-line blocks the PE FIFO
            drain_every[0] = 2
            if level >= 3 and not os.environ.get("K_NOQUANTA"):
                if pos in (0, 2):
                    j = 1 if pos == 0 else 2
                    pending.append(lambda j=j: emit_qk(j, wq_sb, q_nat))
                    pending.append(lambda j=j: emit_qk(j, wk_sb, k_nat))
                    pending.append(lambda j=j: emit_v(j, 0))
                    pending.append(lambda j=j: emit_v(j, 1))
                elif pos == 3:
                    pending.append(lambda: emit_qk(3, wq_sb, q_nat))
                    pending.append(lambda: emit_qk(3, wk_sb, k_nat))
                elif pos == 4:
                    pending.append(lambda: emit_v(3, 0))
                    pending.append(lambda: emit_v(3, 1))
            if level >= 5:
                po_sched = {
                    3: (0, 1),           # chunk 0 (processed at pos 1)
                    4: (2, 3),           # chunk 1
                    5: (4, 5, 6, 7, 8, 9),   # chunks 2,3,4
                    6: (10, 11),         # chunk 5
                    7: (14, 15),         # chunk 7 (processed at pos 6)
                }
                for g in po_sched.get(pos, ()):
                    pending.append(lambda g=g: emit_po(g))

            nk = 2 * a + 2
            asum = ps_acc.tile([128, HPC, WA], FP32, tag="acc", name="asum")

            def S(i, a=a):
                sct = ps_sc.tile([128, HPC, WA], FP32, tag="sc", name="sct")
                # packed pairs: heads 2p / 2p+1 stream concurrently through
                # PE row-groups 0:64 / 64:128 into different psum banks.
                # start=True only on each bank's first writer (h0 -> bank0,
                # h1 -> bank1); the second writer overwrites fresh since the
                # bank's has_written bits were cleared.
                for h in range(HPC):
                    p, half = h // 2, h % 2
                    nc.tensor.matmul(
                        sct[:, SLOT[h], :],
                        lhsT=k_nat[p][ds(64 * half, 64), ts(i, 128)],
                        rhs=q_nat[p][ds(64 * half, 64), ts(a, WA)],
                        start=(h < 2),
                        stop=True,
                        tile_position=(64 * half, 0),
                        skip_group_check=True,
                    )
                return sct

            def EPV(i, sct, nk=nk, asum=asum, a=a):
                pr = probs_p.tile([128, HPC, WA], BF16, tag="pr", name="pr")
                nc.scalar.activation(
                    pr[:], sct[:], mybir.ActivationFunctionType.Exp, scale=SCALE
                )
                if i >= 2 * a:
                    # diagonal k-tile: zero the masked entries on the DVE
                    # (exp(s)*mask == softmax numerator with -inf masking)
                    nc.vector.tensor_mul(pr[:], pr[:], dm_sb[:, i - 2 * a, :, :])
                for h in range(HPC):
                    # i==0, even h: start=True clears the whole bank's
                    # has_written bits; the odd head's first matmul then
                    # overwrites (its bits are clear) — no zero-init needed.
                    nc.tensor.matmul(
                        asum[:, h, :],
                        lhsT=v_aug[:, i, h, :],
                        rhs=pr[:, SLOT[h], :],
                        start=(i == 0 and h % 2 == 0),
                        stop=(i == nk - 1),
                        skip_group_check=True,
                    )

            prev = None
            for i in range(nk):
                sct = S(i)
                if prev is not None:
                    EPV(prev[0], prev[1])
                prev = (i, sct)
                gap[0] += 1
                drain_one()
                if pos < 3:
                    # HAM keep-warm trickle: dummy LDWEIGHTS during the
                    # early dependency gaps keep the PE activity monitor
                    # from re-throttling the clock to 1.2 GHz (two are
                    # needed — one alone measurably fails to hold K=8/8).
                    # Harmless: every real matmul reloads its own weights.
                    # pos 3-4 are quanta-dense: trickles there cost more
                    # inline time than the cold they prevent.
                    nc.tensor.ldweights(weights=zdum[:])
                    nc.tensor.ldweights(weights=zdum[:])
            EPV(prev[0], prev[1])
            # chunk-boundary trickle: the epilogue hand-off can idle the PE
            # past the HAM window; two dummy LDWEIGHTS hold the clock warm
            nc.tensor.ldweights(weights=zdum[:])
            nc.tensor.ldweights(weights=zdum[:])

            if os.environ.get("K_NOEPI"):
                continue
            if pos == n_pos - 1 and not os.environ.get("K_NOFASTEPI"):
                # tail fast path: stage only the DENOMINATOR halves to SBUF
                # (half the copy bytes), swap, reciprocal, and multiply
                # reading the numerators straight from psum — ~2us shorter
                # chain before the last outproj tiles can start.
                sd = asb_p.tile([128, 2, WA], FP32, tag="asb", name="sd")
                den = den_p.tile([128, 2, WA], FP32, tag="den", name="den")
                rc = den_p.tile([128, 2, WA], FP32, tag="rc", name="rc")
                nc.vector.tensor_copy(sd[ds(64, 64), :, :], asum[ds(64, 64), 0:4:2, :])
                nc.scalar.copy(sd[ds(0, 64), :, :], asum[ds(0, 64), 1:4:2, :])
                nc.sync.dma_start(den[ds(0, 64), :, :], sd[ds(64, 64), :, :])
                nc.sync.dma_start(den[ds(64, 64), :, :], sd[ds(0, 64), :, :])
                nc.vector.reciprocal_approx_fast(rc[:], den[:])
                for p in range(2):
                    nc.vector.tensor_mul(
                        attn_nrm[p][ds(0, 64), ts(a, WA)],
                        asum[ds(0, 64), 2 * p, :],
                        rc[ds(0, 64), p, :],
                    )
                    nc.vector.tensor_mul(
                        attn_nrm[p][ds(64, 64), ts(a, WA)],
                        asum[ds(64, 64), 2 * p + 1, :],
                        rc[ds(64, 64), p, :],
                    )
                continue
            # epilogue: drain asum per bank-pair (vector), den swap per pair
            # right behind its copy, reciprocal + normalize per pair — the
            # pair-0 chain completes ~1us earlier than a monolithic drain.
            asb = asb_p.tile([128, HPC, WA], FP32, tag="asb", name="asb")
            den = den_p.tile([128, 2, WA], FP32, tag="den", name="den")
            rc = den_p.tile([128, 2, WA], FP32, tag="rc", name="rc")
            # in the fill phase (and the tail, where the exp stream has
            # ended) the DVE is the bottleneck — scalar takes the asum
            # drains there so the reciprocal+normalize chain starts sooner
            drain_scalar = pos < 3 or pos == n_pos - 1
            if drain_scalar:
                nc.scalar.copy(asb[:, 0:2, :], asum[:, 0:2, :])
            else:
                nc.vector.tensor_copy(asb[:, 0:2, :], asum[:, 0:2, :])
            nc.sync.dma_start(den[ds(0, 64), 0, :], asb[ds(64, 64), 0, :])
            nc.sync.dma_start(den[ds(64, 64), 0, :], asb[ds(0, 64), 1, :])
            if drain_scalar:
                nc.scalar.copy(asb[:, 2:4, :], asum[:, 2:4, :])
            else:
                nc.vector.tensor_copy(asb[:, 2:4, :], asum[:, 2:4, :])
            nc.sync.dma_start(den[ds(0, 64), 1, :], asb[ds(64, 64), 2, :])
            nc.sync.dma_start(den[ds(64, 64), 1, :], asb[ds(0, 64), 3, :])
            for p in range(2):
                nc.vector.reciprocal_approx_fast(rc[:, p, :], den[:, p, :])
                nc.vector.tensor_mul(
                    attn_nrm[p][ds(0, 64), ts(a, WA)],
                    asb[ds(0, 64), 2 * p, :],
                    rc[ds(0, 64), p, :],
                )
                nc.vector.tensor_mul(
                    attn_nrm[p][ds(64, 64), ts(a, WA)],
                    asb[ds(64, 64), 2 * p + 1, :],
                    rc[ds(64, 64), p, :],
                )

        # tail: whatever quanta remain + chunk 6's output tiles
        for f in pending:
            f()
        if level >= 5:
            # tail tiles go in the now-idle scores pool so they run in
            # parallel instead of serializing on the single-buffer io ring
            for g in (12, 13):
                emit_po(g, pool=ps_sc, tag="sc", tail=True)

    nc.compile()
    return nc


def make_consts(cos, sin):
    cosT = np.ascontiguousarray(
        np.tile(np.asarray(cos[0], dtype=np.float32).T[:32], (4, 1))
    ).astype(ml_dtypes.bfloat16)
    sinT = np.ascontiguousarray(
        np.tile(np.asarray(sin[0], dtype=np.float32).T[:32], (4, 1))
    ).astype(ml_dtypes.bfloat16)
    # dmask[kp, idx*HPC*WA + h*WA + qc] = 1 if (128*idx + kp) <= qc else 0
    kp = np.arange(128)[:, None]
    qc = np.arange(WA)[None, :]
    dm = np.stack(
        [
            np.repeat(((128 * idx + kp) <= qc)[:, None, :], HPC, axis=1)
            for idx in range(2)
        ],
        axis=1,
    )  # [128, 2, HPC, WA]
    dmask = np.ascontiguousarray(
        dm.reshape(128, 2 * HPC * WA).astype(ml_dtypes.bfloat16)
    )
    return dict(cosT=cosT, sinT=sinT, dmask=dmask)


def host_prep(core, xT_by_batch, Wq, Wk, Wv, Wp, consts):
    b, hp = core // 4, core % 4
    h0 = hp * HPC
    rows = slice(HD * h0, HD * h0 + E)
    Wq_s = np.asarray(Wq[rows]).reshape(HPC, HD, D)
    Wk_s = np.asarray(Wk[rows]).reshape(HPC, HD, D)
    wqT = np.ascontiguousarray(
        np.concatenate(
            [Wq_s[:, :32].reshape(128, D), Wq_s[:, 32:].reshape(128, D)], 0
        ).T.astype(ml_dtypes.bfloat16)
    )
    wkT = np.ascontiguousarray(
        np.concatenate(
            [Wk_s[:, :32].reshape(128, D), Wk_s[:, 32:].reshape(128, D)], 0
        ).T.astype(ml_dtypes.bfloat16)
    )
    wvT = np.ascontiguousarray(np.asarray(Wv[rows]).T.astype(ml_dtypes.bfloat16))
    wpT = np.ascontiguousarray(np.asarray(Wp[:, rows]).T.astype(ml_dtypes.bfloat16))
    return dict(
        xT_b=xT_by_batch[b],
        wqT=wqT,
        wkT=wkT,
        wvT=wvT,
        wpT=wpT,
        **consts,
    )


_NC_CACHE = None


def _get_nc():
    global _NC_CACHE
    if _NC_CACHE is None:
        _NC_CACHE = build_program()
    return _NC_CACHE


def kernel(x, cos, sin, Wq, Wk, Wv, Wp, _want_trace=False):
    x, cos, sin = np.asarray(x), np.asarray(cos), np.asarray(sin)
    Wq, Wk, Wv, Wp = (np.asarray(a) for a in (Wq, Wk, Wv, Wp))
    nc = _get_nc()
    consts = make_consts(cos, sin)
    xT_by_batch = [
        np.ascontiguousarray(x[b].T.astype(ml_dtypes.bfloat16)) for b in range(B)
    ]
    in_maps = [
        host_prep(core, xT_by_batch, Wq, Wk, Wv, Wp, consts) for core in range(8)
    ]
    res = run_bass_kernel_spmd(nc, in_maps, list(range(8)), trace=_want_trace)
    out = np.zeros((B, T, D), dtype=np.float32)
    for core in range(8):
        out[core // 4] += np.asarray(res.results[core]["outp"], dtype=np.float32)
    if _want_trace:
        kernel.last_exec_time_ns = res.exec_time_ns
        kernel.last_profile = res.profile_json
    return out


# revision 67
# speedup vs baseline: 1.0204x; 1.0101x over previous
"""Trainium2 Bass kernel for nn_MultiHeadAttention (B=2,T=2048,D=1024,H=16,HD=64).

Sharding: 8 cores = 2 batches x 4 heads/core (tensor parallel over heads).
Each core computes q,k,v projections for its 4 heads, RoPE, causal
flash-attention, and a partial output projection (its heads' slice of Wp);
the host sums the 4 partials per batch.

v3 design (on top of the fully-pipelined v2), measured 223us -> ~194us:
  - Packed scores matmuls: q/k stored as head-PAIR tiles [128, T]; each
    pair's two 64-row scores matmuls run CONCURRENTLY in the two PE
    array row-group halves (tile_position (0,0)/(64,0)) into different
    psum banks — halves scores wall time.
  - Causal diagonal masking moved off the PE: exp output is multiplied
    by a precomputed 0/1 tile on the DVE (kills 64 mask matmuls + the
    u/l mask weights).
  - PE HAM warm-up: dummy matmuls during the initial DMA wait plus a
    2x dummy-LDWEIGHTS trickle per early iteration hold the PE clock at
    2.4 GHz through the fill phase.
  - Startup: input DMAs split into column chunks and priority-ordered
    across the three DMA queues (sync-HW, scalar-HW, gpsimd-SW) so the
    first projection matmul starts at ~10.5us instead of ~19us.
  - exp activation table prefetched with a dummy 1-element exp at t=0.
  - Chunk order [1,0,2,3,4,5,7,6]: chunk 0 (inputs already resident)
    fills the xT j=1 DMA window; small tail chain hangs off chunk 6
    with a fast-path epilogue (reciprocal+normalize read psum direct).
  - Zero-init matmuls for the PV accumulator removed: the first PV
    matmul of each psum bank uses start=True (clears the whole bank's
    has_written bits; the co-banked head's first matmul then overwrites
    since its bits are clear).
  - q/k RoPE'd tiles stored with lo/hi rows interleaved (one combined
    DMA per head instead of two): scores are invariant to any row
    permutation applied consistently to q and k.
  - Output stored bf16 (host accumulates partials in fp32); tail
    stores pipelined per 512-col half on the HW DGE queues.
"""

import os
import sys

sys.path.insert(0, "/opt/trn_rl_repo")

from contextlib import ExitStack

import numpy as np
import ml_dtypes

import concourse.bass as bass
import concourse.bacc as bacc
import concourse.tile as tile
import concourse.mybir as mybir
from concourse.bass import ts, ds
from concourse.bass_utils import run_bass_kernel_spmd

B, T, D, H, HD = 2, 2048, 1024, 16, 64
HPC = 4                # heads per core
E = HPC * HD           # 256 per-core channels
WP = 512               # projection chunk width (t)
WA = 256               # attention chunk width (q)
NPC = T // WP          # 4
NAC = T // WA          # 8
NKT = T // 128         # 16 k-tiles
DQ = D // 128          # 8 contraction subtiles
NEG = -10000.0
FP32 = mybir.dt.float32
BF16 = mybir.dt.bfloat16
SCALE = 1.0 / float(np.sqrt(HD))
NTT = T // 128         # 16 t-tiles for the output projection


def build_program(level=99):
    nc = bacc.Bacc("TRN2", target_bir_lowering=False, debug=False)
    xT_in = nc.declare_dram_parameter("xT_b", [D, T], BF16, isOutput=False)
    wqT = nc.declare_dram_parameter("wqT", [D, E], BF16, isOutput=False)
    wkT = nc.declare_dram_parameter("wkT", [D, E], BF16, isOutput=False)
    wvT = nc.declare_dram_parameter("wvT", [D, E], BF16, isOutput=False)
    wpT = nc.declare_dram_parameter("wpT", [E, D], BF16, isOutput=False)
    cosT = nc.declare_dram_parameter("cosT", [128, T], BF16, isOutput=False)
    sinT = nc.declare_dram_parameter("sinT", [128, T], BF16, isOutput=False)
    # dmask[:, idx, h, :] = causal 0/1 mask for the two diagonal k-tile
    # positions (idx 0: k-tile aligned with chunk start, idx 1: +128),
    # replicated over the 4 head slots.
    dmask = nc.declare_dram_parameter("dmask", [128, 2 * HPC * WA], BF16, isOutput=False)
    outp = nc.declare_dram_parameter("outp", [T, D], BF16, isOutput=True)

    with tile.TileContext(nc) as tc, ExitStack() as ctx:
        consts = ctx.enter_context(tc.tile_pool(name="consts", bufs=1))
        ropet = ctx.enter_context(tc.tile_pool(name="ropet", bufs=2))
        probs_p = ctx.enter_context(
            tc.tile_pool(name="probs", bufs=int(os.environ.get("K_PRBUFS", "3")))
        )
        asb_p = ctx.enter_context(tc.tile_pool(name="asb", bufs=2))
        den_p = ctx.enter_context(tc.tile_pool(name="den", bufs=2))
        ostage = ctx.enter_context(tc.tile_pool(name="ostage", bufs=2))
        warm_p = ctx.enter_context(tc.tile_pool(name="warm", bufs=1))
        ps_sc = ctx.enter_context(
            tc.tile_pool(
                name="ps_sc", bufs=int(os.environ.get("K_SCBUFS", "2")), space="PSUM"
            )
        )
        ps_acc = ctx.enter_context(tc.tile_pool(name="ps_acc", bufs=1, space="PSUM"))
        ps_io = ctx.enter_context(tc.tile_pool(name="ps_io", bufs=1, space="PSUM"))

        # ---- exp table prefetch: 1-element dummy activation at t~0 ----
        warm = warm_p.tile([1, 8], FP32, tag="warm")
        nc.vector.memset(warm[:, 0:4], 0.0)
        nc.scalar.activation(
            warm[:, 4:8], warm[:, 0:4], mybir.ActivationFunctionType.Exp, scale=1.0
        )
        # ---- PE HAM warm-up: dummy matmuls during the input-DMA wait ----
        # The PE clock-gate (HAM) needs ~3.4us of sustained matmul activity
        # to un-throttle from 1.2 to 2.4 GHz.  The input DMAs take ~11us,
        # so without this the whole prologue runs at half clock.  Dummy
        # N=128 matmuls on a memset tile keep the PE busy until real data
        # lands; they are ahead of the real work in the PE FIFO and finish
        # just before it becomes ready.
        zdum = warm_p.tile([128, 128], BF16, tag="zdum")
        nc.vector.memset(zdum[:], 0.0)
        # ~26 dummies run cold before HAM un-throttles (~3.4us); a few more
        # keep it warm until wq lands (~10.5us).  More than that delays the
        # first real LDWEIGHTS behind the dummy stream.
        n_warm = int(os.environ.get("K_WARM_MM", "40"))
        if n_warm:
            pwarm = ps_sc.tile([128, 128], FP32, tag="sc", name="pwarm")
            for _ in range(n_warm):
                nc.tensor.matmul(
                    pwarm[:],
                    lhsT=zdum[:],
                    rhs=zdum[:],
                    start=True,
                    stop=True,
                    skip_group_check=True,
                )

        # ---- constants / weights / x to SBUF, priority-ordered ----
        # Queue assignment (3 parallel DMA paths): sync=HW, scalar=HW,
        # gpsimd=SW.  Critical prefix: wq; xT j=0; wk; cos/sin j=0.
        xT_sb = consts.tile([128, DQ, T], BF16, tag="xT")
        xT_r = xT_in.rearrange("(o p) m -> p o m", p=128)
        wq_sb = consts.tile([128, DQ, E], BF16, tag="wq")
        wq_r = wqT.rearrange("(o p) m -> p o m", p=128)
        wk_sb = consts.tile([128, DQ, E], BF16, tag="wk")
        wk_r = wkT.rearrange("(o p) m -> p o m", p=128)
        wv_sb = consts.tile([128, DQ, E], BF16, tag="wv")
        wv_r = wvT.rearrange("(o p) m -> p o m", p=128)
        wp_sb = consts.tile([128, 2, D], BF16, tag="wp")
        cos_sb = consts.tile([128, T], BF16, tag="cos")
        sin_sb = consts.tile([128, T], BF16, tag="sin")
        dm_sb = consts.tile([128, 2, HPC, WA], BF16, tag="dmask")

        # wave 0: the prologue's critical inputs, spread across all three
        # DMA queues: wq (sync), xT j0 (all queues), cos/sin j0 (scalar).
        nc.sync.dma_start(wq_sb[:, 0:4, :], wq_r[:, 0:4, :])
        nc.sync.dma_start(wq_sb[:, 4:8, :], wq_r[:, 4:8, :])
        nc.scalar.dma_start(cos_sb[:, ts(0, WP)], cosT[:, ts(0, WP)])
        nc.scalar.dma_start(sin_sb[:, ts(0, WP)], sinT[:, ts(0, WP)])
        for dq in range(3):
            nc.scalar.dma_start(xT_sb[:, dq, ts(0, WP)], xT_r[:, dq, ts(0, WP)])
        for dq in range(3, 6):
            nc.gpsimd.dma_start(xT_sb[:, dq, ts(0, WP)], xT_r[:, dq, ts(0, WP)])
        for dq in range(6, 8):
            nc.sync.dma_start(xT_sb[:, dq, ts(0, WP)], xT_r[:, dq, ts(0, WP)])
        # wave 1: wk, wv, and xT j1 balanced across the queues so the j=1
        # projection quantum (drained during chunk 1) never head-of-line
        # blocks the PE on a straggling slice.
        nc.sync.dma_start(wk_sb[:, 0:4, :], wk_r[:, 0:4, :])
        nc.scalar.dma_start(wk_sb[:, 4:8, :], wk_r[:, 4:8, :])
        nc.gpsimd.dma_start(wv_sb[:, 0:4, :], wv_r[:, 0:4, :])
        nc.gpsimd.dma_start(wv_sb[:, 4:8, :], wv_r[:, 4:8, :])
        for dq in range(3):
            nc.scalar.dma_start(xT_sb[:, dq, ts(1, WP)], xT_r[:, dq, ts(1, WP)])
        for dq in range(3, 6):
            nc.gpsimd.dma_start(xT_sb[:, dq, ts(1, WP)], xT_r[:, dq, ts(1, WP)])
        for dq in range(6, 8):
            nc.sync.dma_start(xT_sb[:, dq, ts(1, WP)], xT_r[:, dq, ts(1, WP)])
        nc.scalar.dma_start(dm_sb[:], dmask[:])
        nc.scalar.dma_start(cos_sb[:, ts(1, WP)], cosT[:, ts(1, WP)])
        nc.scalar.dma_start(sin_sb[:, ts(1, WP)], sinT[:, ts(1, WP)])
        # v_aug[:, t, h, :]: even h = [v | ones], odd h = [ones | v]; fill
        # everything with ones, the v copies overwrite their halves.
        # memset on vector: the DVE is idle until the first projection psum
        # lands (~16us), so this is free there, and it keeps the gpsimd
        # engine clear for SW-DGE descriptor generation.
        v_aug = consts.tile([128, NKT, HPC, 128], BF16, tag="vaug")
        nc.vector.memset(v_aug[:], 1.0)

        # wave 2: wp (gpsimd), xT j2 (sync/gpsimd), cos/sin j2-3 (scalar)
        nc.gpsimd.dma_start(wp_sb[:], wpT.rearrange("(o p) m -> p o m", p=128))
        for dq in range(0, DQ, 2):
            nc.sync.dma_start(xT_sb[:, dq, ts(2, WP)], xT_r[:, dq, ts(2, WP)])
            nc.gpsimd.dma_start(
                xT_sb[:, dq + 1, ts(2, WP)], xT_r[:, dq + 1, ts(2, WP)]
            )
        for j in (2, 3):
            nc.scalar.dma_start(cos_sb[:, ts(j, WP)], cosT[:, ts(j, WP)])
            nc.scalar.dma_start(sin_sb[:, ts(j, WP)], sinT[:, ts(j, WP)])
        for dq in range(0, DQ, 2):
            nc.sync.dma_start(xT_sb[:, dq, ts(3, WP)], xT_r[:, dq, ts(3, WP)])
            nc.gpsimd.dma_start(
                xT_sb[:, dq + 1, ts(3, WP)], xT_r[:, dq + 1, ts(3, WP)]
            )

        # q/k pair tiles [128, T]: head 2p on partitions 0:64, head 2p+1 on
        # 64:128.  The scores matmuls for a pair run as two concurrent
        # 64-row PE tiles (tile_position (0,0) / (64,0)) — both halves of
        # the systolic array stream at once, halving scores wall time.
        q_nat = [
            consts.tile([128, T], BF16, tag=f"qnat{p}", name=f"qnat{p}")
            for p in range(2)
        ]
        k_nat = [
            consts.tile([128, T], BF16, tag=f"knat{p}", name=f"knat{p}")
            for p in range(2)
        ]
        # head h -> sct/pr column slot: pair members go to different psum
        # BANKS (h0/h2 in bank 0 = slots 0,1; h1/h3 in bank 1 = slots 2,3)
        # so the two concurrent streams never write the same bank.
        SLOT = [0, 2, 1, 3]
        attn_nrm = [
            consts.tile([128, T], BF16, tag=f"anrm{p}", name=f"anrm{p}")
            for p in range(2)
        ]

        # ---- work quanta (proj / outproj), drained between attn iters ----
        def emit_qk(j, w_sb, nat, dq_order=None, trickle=False):
            pqk = ps_io.tile([128, 2, WP], FP32, tag="io", name="pqk")
            dqs = dq_order if dq_order is not None else range(DQ)
            # dq-major so each matmul only needs its own xT slice (slices
            # arrive one at a time during the prologue); the interleaved
            # bank accumulation groups are fine on HW.
            for n, dq in enumerate(dqs):
                if trickle and n % 2 == 0:
                    # prologue matmuls stall on arriving xT slices; dummy
                    # LDWEIGHTS between them keep the HAM clock-gate warm
                    nc.tensor.ldweights(weights=zdum[:])
                    nc.tensor.ldweights(weights=zdum[:])
                for half in range(2):
                    nc.tensor.matmul(
                        pqk[:, half, :],
                        lhsT=w_sb[:, dq, ds(128 * half, 128)],
                        rhs=xT_sb[:, dq, ts(j, WP)],
                        start=(n == 0),
                        stop=(n == DQ - 1),
                        skip_group_check=True,
                    )
            lo, hi = pqk[:, 0, :], pqk[:, 1, :]
            cs, sn = cos_sb[:, ts(j, WP)], sin_sb[:, ts(j, WP)]
            st = ropet.tile([128, 2, WP], BF16, tag="st", name="st")
            # bf16 intermediates: the final add/sub then runs all-16-bit
            # operands at 2x DVE rate
            ta = ropet.tile([128, 2, WP], BF16, tag="ta", name="ta")
            tb = ropet.tile([128, 2, WP], BF16, tag="tb", name="tb")
            nc.vector.tensor_mul(ta[:, 0, :], lo, cs)
            nc.vector.tensor_mul(ta[:, 1, :], hi, sn)
            nc.vector.tensor_sub(st[:, 0, :], ta[:, 0, :], ta[:, 1, :])
            nc.vector.tensor_mul(tb[:, 0, :], hi, cs)
            nc.vector.tensor_mul(tb[:, 1, :], lo, sn)
            nc.vector.tensor_add(st[:, 1, :], tb[:, 0, :], tb[:, 1, :])
            # one combined DMA per head: rows land interleaved
            # (lo0,hi0,lo1,hi1,...) — scores are invariant to a row
            # permutation applied consistently to q and k.  Head h goes to
            # pair tile h//2, partition half h%2.
            for h in range(HPC):
                nc.sync.dma_start(
                    nat[h // 2][ds(64 * (h % 2), 64), ts(j, WP)],
                    st[ds(32 * h, 32), :, :],
                )

        def emit_v(j, half_pair):
            pv = ps_io.tile([128, 2, E], FP32, tag="io", name="pv")
            for tt in range(2):
                g = 4 * j + 2 * half_pair + tt
                for dq in range(DQ):
                    nc.tensor.matmul(
                        pv[:, tt, :],
                        lhsT=xT_sb[:, dq, ts(g, 128)],
                        rhs=wv_sb[:, dq, :],
                        start=(dq == 0),
                        stop=(dq == DQ - 1),
                    )
            for tt in range(2):
                g = 4 * j + 2 * half_pair + tt
                for h in range(HPC):
                    voff = 0 if h % 2 == 0 else 64
                    nc.vector.tensor_copy(
                        v_aug[:, g, h, ds(voff, 64)], pv[:, tt, ds(64 * h, 64)]
                    )

        def emit_po(g, pool=None, tag="io", tail=False, hwq=False):
            po = (pool or ps_io).tile([128, D], FP32, tag=tag, name="po")
            if tail:
                # tail tiles pipeline per 512-col half: cast + store of the
                # first half run while the second half's matmuls stream.
                # Stores ride the HW DGE queues (sync/scalar); by the tail
                # the exp stream is finished so scalar is free.
                for dh in range(2):
                    for p in range(2):
                        nc.tensor.matmul(
                            po[:, ds(512 * dh, 512)],
                            lhsT=attn_nrm[p][:, ts(g, 128)],
                            rhs=wp_sb[:, p, ds(512 * dh, 512)],
                            start=(p == 0),
                            stop=(p == 1),
                        )
                    ost = ostage.tile([128, 512], BF16, tag="ost", name="ost")
                    if (g + dh) % 2 == 1:
                        nc.scalar.copy(ost[:], po[:, ds(512 * dh, 512)])
                    else:
                        nc.vector.tensor_copy(ost[:], po[:, ds(512 * dh, 512)])
                    deng = nc.scalar if g % 2 == 1 else nc.sync
                    deng.dma_start(
                        outp[ts(g, 128), ds(512 * dh, 512)], ost[:]
                    )
                return
            for dh in range(2):
                for p in range(2):
                    nc.tensor.matmul(
                        po[:, ds(512 * dh, 512)],
                        lhsT=attn_nrm[p][:, ts(g, 128)],
                        rhs=wp_sb[:, p, ds(512 * dh, 512)],
                        start=(p == 0),
                        stop=(p == 1),
                    )
            ost = ostage.tile([128, D], BF16, tag="ost", name="ost")
            nc.vector.tensor_copy(ost[:], po[:])
            if hwq:
                # near-tail stores ride the HW sync queue; the gpsimd SW
                # queue drains too slowly to sit on the exit path
                nc.sync.dma_start(outp[ts(g, 128), :], ost[:])
            elif g % 2 == 0:
                nc.gpsimd.dma_start(outp[ts(g, 128), :], ost[:])
            else:
                nc.sync.dma_start(outp[ts(g, 128), :], ost[:])

        pending = []
        gap = [0]
        drain_every = [1]

        def drain_one():
            if pending and gap[0] >= drain_every[0]:
                pending.pop(0)()
                gap[0] = 0



        # prologue: projection chunk 0 (serial; nothing to overlap with yet)
        # dq consumption ordered by DMA arrival: gpsimd slices (3,4,5) and
        # scalar slices (0,1,2) land before the sync ones (6,7 behind wq).
        if level >= 1 and not os.environ.get("K_NOPRO"):
            arrival = [3, 4, 5, 0, 1, 2, 6, 7]
            emit_qk(0, wq_sb, q_nat, dq_order=arrival, trickle=True)
            emit_qk(0, wk_sb, k_nat, dq_order=arrival, trickle=True)
            emit_v(0, 0)
            emit_v(0, 1)

        # chunk processing order: chunk 0 second — its inputs are already
        # resident, so it fills the PE while the xT j=1 slices land; chunk 6
        # last so chunk 7's outproj tiles drain during it, leaving only
        # g12,13 for the tail.
        order = [1, 0, 2, 3, 4, 5, 7, 6]
        n_pos = {0: 0, 1: 0, 2: 1, 3: 4, 4: NAC}.get(level, NAC)
        if os.environ.get("K_NCHUNKS"):
            n_pos = int(os.environ["K_NCHUNKS"])
        # ---- attention chunks, with quanta interleaved ----
        for pos in range(n_pos):
            a = order[pos]
            # draining faster than every 2 iters front-loads quanta whose
            # input DMAs haven't landed and head-of-line blocks the PE FIFO
            drain_every[0] = 2
            if level >= 3 and not os.environ.get("K_NOQUANTA"):
                if pos in (0, 2):
                    j = 1 if pos == 0 else 2
                    pending.append(lambda j=j: emit_qk(j, wq_sb, q_nat))
                    pending.append(lambda j=j: emit_qk(j, wk_sb, k_nat))
                    pending.append(lambda j=j: emit_v(j, 0))
                    pending.append(lambda j=j: emit_v(j, 1))
                elif pos == 3:
                    pending.append(lambda: emit_qk(3, wq_sb, q_nat))
                    pending.append(lambda: emit_qk(3, wk_sb, k_nat))
                elif pos == 4:
                    pending.append(lambda: emit_v(3, 0))
                    pending.append(lambda: emit_v(3, 1))
            if level >= 5:
                po_sched = {
                    3: (0, 1),           # chunk 0 (processed at pos 1)
                    4: (2, 3),           # chunk 1
                    5: (4, 5, 6, 7, 8, 9),   # chunks 2,3,4
                    6: (10, 11),         # chunk 5
                    7: (14, 15),         # chunk 7 (processed at pos 6)
                }
                for g in po_sched.get(pos, ()):
                    pending.append(lambda g=g: emit_po(g))

            nk = 2 * a + 2
            asum = ps_acc.tile([128, HPC, WA], FP32, tag="acc", name="asum")

            def S(i, a=a):
                sct = ps_sc.tile([128, HPC, WA], FP32, tag="sc", name="sct")
                # packed pairs: heads 2p / 2p+1 stream concurrently through
                # PE row-groups 0:64 / 64:128 into different psum banks.
                # start=True only on each bank's first writer (h0 -> bank0,
                # h1 -> bank1); the second writer overwrites fresh since the
                # bank's has_written bits were cleared.
                for h in range(HPC):
                    p, half = h // 2, h % 2
                    nc.tensor.matmul(
                        sct[:, SLOT[h], :],
                        lhsT=k_nat[p][ds(64 * half, 64), ts(i, 128)],
                        rhs=q_nat[p][ds(64 * half, 64), ts(a, WA)],
                        start=(h < 2),
                        stop=True,
                        tile_position=(64 * half, 0),
                        skip_group_check=True,
                    )
                return sct

            def EPV(i, sct, nk=nk, asum=asum, a=a):
                pr = probs_p.tile([128, HPC, WA], BF16, tag="pr", name="pr")
                nc.scalar.activation(
                    pr[:], sct[:], mybir.ActivationFunctionType.Exp, scale=SCALE
                )
                if i >= 2 * a:
                    # diagonal k-tile: zero the masked entries on the DVE
                    # (exp(s)*mask == softmax numerator with -inf masking)
                    nc.vector.tensor_mul(pr[:], pr[:], dm_sb[:, i - 2 * a, :, :])
                for h in range(HPC):
                    # i==0, even h: start=True clears the whole bank's
                    # has_written bits; the odd head's first matmul then
                    # overwrites (its bits are clear) — no zero-init needed.
                    nc.tensor.matmul(
                        asum[:, h, :],
                        lhsT=v_aug[:, i, h, :],
                        rhs=pr[:, SLOT[h], :],
                        start=(i == 0 and h % 2 == 0),
                        stop=(i == nk - 1),
                        skip_group_check=True,
                    )

            prev = None
            for i in range(nk):
                sct = S(i)
                if prev is not None:
                    EPV(prev[0], prev[1])
                prev = (i, sct)
                gap[0] += 1
                drain_one()
                if pos < 3:
                    # HAM keep-warm trickle: dummy LDWEIGHTS during the
                    # early dependency gaps keep the PE activity monitor
                    # from re-throttling the clock to 1.2 GHz (two are
                    # needed — one alone measurably fails to hold K=8/8).
                    # Harmless: every real matmul reloads its own weights.
                    # pos 3-4 are quanta-dense: trickles there cost more
                    # inline time than the cold they prevent.
                    nc.tensor.ldweights(weights=zdum[:])
                    nc.tensor.ldweights(weights=zdum[:])
            EPV(prev[0], prev[1])
            # chunk-boundary trickle: the epilogue hand-off can idle the PE
            # past the HAM window; two dummy LDWEIGHTS hold the clock warm
            nc.tensor.ldweights(weights=zdum[:])
            nc.tensor.ldweights(weights=zdum[:])

            if os.environ.get("K_NOEPI"):
                continue
            if pos == n_pos - 1 and not os.environ.get("K_NOFASTEPI"):
                # tail fast path: stage only the DENOMINATOR halves to SBUF
                # (half the copy bytes), swap, reciprocal, and multiply
                # reading the numerators straight from psum — ~2us shorter
                # chain before the last outproj tiles can start.
                sd = asb_p.tile([128, 2, WA], FP32, tag="asb", name="sd")
                den = den_p.tile([128, 2, WA], FP32, tag="den", name="den")
                rc = den_p.tile([128, 2, WA], FP32, tag="rc", name="rc")
                nc.vector.tensor_copy(sd[ds(64, 64), :, :], asum[ds(64, 64), 0:4:2, :])
                nc.scalar.copy(sd[ds(0, 64), :, :], asum[ds(0, 64), 1:4:2, :])
                nc.sync.dma_start(den[ds(0, 64), :, :], sd[ds(64, 64), :, :])
                nc.sync.dma_start(den[ds(64, 64), :, :], sd[ds(0, 64), :, :])
                nc.vector.reciprocal_approx_fast(rc[:], den[:])
                for p in range(2):
                    nc.vector.tensor_mul(
                        attn_nrm[p][ds(0, 64), ts(a, WA)],
                        asum[ds(0, 64), 2 * p, :],
                        rc[ds(0, 64), p, :],
                    )
                    nc.vector.tensor_mul(
                        attn_nrm[p][ds(64, 64), ts(a, WA)],
                        asum[ds(64, 64), 2 * p + 1, :],
                        rc[ds(64, 64), p, :],
                    )
                continue
            # epilogue: drain asum per bank-pair (vector), den swap per pair
            # right behind its copy, reciprocal + normalize per pair — the
            # pair-0 chain completes ~1us earlier than a monolithic drain.
            asb = asb_p.tile([128, HPC, WA], FP32, tag="asb", name="asb")
            den = den_p.tile([128, 2, WA], FP32, tag="den", name="den")
            rc = den_p.tile([128, 2, WA], FP32, tag="rc", name="rc")
            # in the fill phase (and the tail, where the exp stream has
            # ended) the DVE is the bottleneck — scalar takes the asum
            # drains there so the reciprocal+normalize chain starts sooner
            drain_scalar = pos < 3 or pos == n_pos - 1
            if drain_scalar:
                nc.scalar.copy(asb[:, 0:2, :], asum[:, 0:2, :])
            else:
                nc.vector.tensor_copy(asb[:, 0:2, :], asum[:, 0:2, :])
            nc.sync.dma_start(den[ds(0, 64), 0, :], asb[ds(64, 64), 0, :])
            nc.sync.dma_start(den[ds(64, 64), 0, :], asb[ds(0, 64), 1, :])
            if drain_scalar:
                nc.scalar.copy(asb[:, 2:4, :], asum[:, 2:4, :])
            else:
                nc.vector.tensor_copy(asb[:, 2:4, :], asum[:, 2:4, :])
            nc.sync.dma_start(den[ds(0, 64), 1, :], asb[ds(64, 64), 2, :])
            nc.sync.dma_start(den[ds(64, 64), 1, :], asb[ds(0, 64), 3, :])
            for p in range(2):
                nc.vector.reciprocal_approx_fast(rc[:, p, :], den[:, p, :])
                nc.vector.tensor_mul(
                    attn_nrm[p][ds(0, 64), ts(a, WA)],
                    asb[ds(0, 64), 2 * p, :],
                    rc[ds(0, 64), p, :],
                )
                nc.vector.tensor_mul(
                    attn_nrm[p][ds(64, 64), ts(a, WA)],
                    asb[ds(64, 64), 2 * p + 1, :],
                    rc[ds(64, 64), p, :],
                )

        # tail: whatever quanta remain + chunk 6's output tiles
        for f in pending:
            f()
        if level >= 5:
            # tail tiles go in the now-idle scores pool so they run in
            # parallel instead of serializing on the single-buffer io ring
            for g in (12, 13):
                emit_po(g, pool=ps_sc, tag="sc", tail=True)

    nc.compile()
    return nc


def make_consts(cos, sin):
    cosT = np.ascontiguousarray(
        np.tile(np.asarray(cos[0], dtype=np.float32).T[:32], (4, 1))
    ).astype(ml_dtypes.bfloat16)
    sinT = np.ascontiguousarray(
        np.tile(np.asarray(sin[0], dtype=np.float32).T[:32], (4, 1))
    ).astype(ml_dtypes.bfloat16)
    # dmask[kp, idx*HPC*WA + h*WA + qc] = 1 if (128*idx + kp) <= qc else 0
    kp = np.arange(128)[:, None]
    qc = np.arange(WA)[None, :]
    dm = np.stack(
        [
            np.repeat(((128 * idx + kp) <= qc)[:, None, :], HPC, axis=1)
            for idx in range(2)
        ],
        axis=1,
    )  # [128, 2, HPC, WA]
    dmask = np.ascontiguousarray(
        dm.reshape(128, 2 * HPC * WA).astype(ml_dtypes.bfloat16)
    )
    return dict(cosT=cosT, sinT=sinT, dmask=dmask)


def host_prep(core, xT_by_batch, Wq, Wk, Wv, Wp, consts):
    b, hp = core // 4, core % 4
    h0 = hp * HPC
    rows = slice(HD * h0, HD * h0 + E)
    Wq_s = np.asarray(Wq[rows]).reshape(HPC, HD, D)
    Wk_s = np.asarray(Wk[rows]).reshape(HPC, HD, D)
    wqT = np.ascontiguousarray(
        np.concatenate(
            [Wq_s[:, :32].reshape(128, D), Wq_s[:, 32:].reshape(128, D)], 0
        ).T.astype(ml_dtypes.bfloat16)
    )
    wkT = np.ascontiguousarray(
        np.concatenate(
            [Wk_s[:, :32].reshape(128, D), Wk_s[:, 32:].reshape(128, D)], 0
        ).T.astype(ml_dtypes.bfloat16)
    )
    wvT = np.ascontiguousarray(np.asarray(Wv[rows]).T.astype(ml_dtypes.bfloat16))
    wpT = np.ascontiguousarray(np.asarray(Wp[:, rows]).T.astype(ml_dtypes.bfloat16))
    return dict(
        xT_b=xT_by_batch[b],
        wqT=wqT,
        wkT=wkT,
        wvT=wvT,
        wpT=wpT,
        **consts,
    )


_NC_CACHE = None


def _get_nc():
    global _NC_CACHE
    if _NC_CACHE is None:
        _NC_CACHE = build_program()
    return _NC_CACHE


def kernel(x, cos, sin, Wq, Wk, Wv, Wp, _want_trace=False):
    x, cos, sin = np.asarray(x), np.asarray(cos), np.asarray(sin)
    Wq, Wk, Wv, Wp = (np.asarray(a) for a in (Wq, Wk, Wv, Wp))
    nc = _get_nc()
    consts = make_consts(cos, sin)
    xT_by_batch = [
        np.ascontiguousarray(x[b].T.astype(ml_dtypes.bfloat16)) for b in range(B)
    ]
    in_maps = [
        host_prep(core, xT_by_batch, Wq, Wk, Wv, Wp, consts) for core in range(8)
    ]
    res = run_bass_kernel_spmd(nc, in_maps, list(range(8)), trace=_want_trace)
    out = np.zeros((B, T, D), dtype=np.float32)
    for core in range(8):
        out[core // 4] += np.asarray(res.results[core]["outp"], dtype=np.float32)
    if _want_trace:
        kernel.last_exec_time_ns = res.exec_time_ns
        kernel.last_profile = res.profile_json
    return out


# revision 68
# speedup vs baseline: 1.0314x; 1.0108x over previous
"""Trainium2 Bass kernel for nn_MultiHeadAttention (B=2,T=2048,D=1024,H=16,HD=64).

Sharding: 8 cores = 2 batches x 4 heads/core (tensor parallel over heads).
Each core computes q,k,v projections for its 4 heads, RoPE, causal
flash-attention, and a partial output projection (its heads' slice of Wp);
the host sums the 4 partials per batch.

v3 design (on top of the fully-pipelined v2), measured 223us -> ~193us:
  - Packed scores matmuls: q/k stored as head-PAIR tiles [128, T]; each
    pair's two 64-row scores matmuls run CONCURRENTLY in the two PE
    array row-group halves (tile_position (0,0)/(64,0)) into different
    psum banks — halves scores wall time.
  - Causal diagonal masking moved off the PE: exp output is multiplied
    by a precomputed 0/1 tile on the DVE (kills 64 mask matmuls + the
    u/l mask weights).
  - PE HAM warm-up: dummy matmuls during the initial DMA wait plus a
    2x dummy-LDWEIGHTS trickle per early iteration hold the PE clock at
    2.4 GHz through the fill phase.
  - Startup: input DMAs split into column chunks and priority-ordered
    across the three DMA queues (sync-HW, scalar-HW, gpsimd-SW) so the
    first projection matmul starts at ~10.5us instead of ~19us.
  - exp activation table prefetched with a dummy 1-element exp at t=0.
  - Chunk order [1,0,2,3,4,5,7,6]: chunk 0 (inputs already resident)
    fills the xT j=1 DMA window; small tail chain hangs off chunk 6
    with a fast-path epilogue (reciprocal+normalize read psum direct).
  - Zero-init matmuls for the PV accumulator removed: the first PV
    matmul of each psum bank uses start=True (clears the whole bank's
    has_written bits; the co-banked head's first matmul then overwrites
    since its bits are clear).
  - q/k RoPE'd tiles stored with lo/hi rows interleaved (one combined
    DMA per head instead of two): scores are invariant to any row
    permutation applied consistently to q and k.
  - Output stored bf16 (host accumulates partials in fp32); tail
    stores pipelined per 512-col half on the HW DGE queues.
"""

import os
import sys

sys.path.insert(0, "/opt/trn_rl_repo")

from contextlib import ExitStack

import numpy as np
import ml_dtypes

import concourse.bass as bass
import concourse.bacc as bacc
import concourse.tile as tile
import concourse.mybir as mybir
from concourse.bass import ts, ds
from concourse.bass_utils import run_bass_kernel_spmd

B, T, D, H, HD = 2, 2048, 1024, 16, 64
HPC = 4                # heads per core
E = HPC * HD           # 256 per-core channels
WP = 512               # projection chunk width (t)
WA = 256               # attention chunk width (q)
NPC = T // WP          # 4
NAC = T // WA          # 8
NKT = T // 128         # 16 k-tiles
DQ = D // 128          # 8 contraction subtiles
NEG = -10000.0
FP32 = mybir.dt.float32
BF16 = mybir.dt.bfloat16
SCALE = 1.0 / float(np.sqrt(HD))
NTT = T // 128         # 16 t-tiles for the output projection


def build_program(level=99):
    nc = bacc.Bacc("TRN2", target_bir_lowering=False, debug=False)
    xT_in = nc.declare_dram_parameter("xT_b", [D, T], BF16, isOutput=False)
    wqT = nc.declare_dram_parameter("wqT", [D, E], BF16, isOutput=False)
    wkT = nc.declare_dram_parameter("wkT", [D, E], BF16, isOutput=False)
    wvT = nc.declare_dram_parameter("wvT", [D, E], BF16, isOutput=False)
    wpT = nc.declare_dram_parameter("wpT", [E, D], BF16, isOutput=False)
    cosT = nc.declare_dram_parameter("cosT", [128, T], BF16, isOutput=False)
    sinT = nc.declare_dram_parameter("sinT", [128, T], BF16, isOutput=False)
    # dmask[:, idx, h, :] = causal 0/1 mask for the two diagonal k-tile
    # positions (idx 0: k-tile aligned with chunk start, idx 1: +128),
    # replicated over the 4 head slots.
    dmask = nc.declare_dram_parameter("dmask", [128, 2 * HPC * WA], BF16, isOutput=False)
    outp = nc.declare_dram_parameter("outp", [T, D], BF16, isOutput=True)

    with tile.TileContext(nc) as tc, ExitStack() as ctx:
        consts = ctx.enter_context(tc.tile_pool(name="consts", bufs=1))
        ropet = ctx.enter_context(tc.tile_pool(name="ropet", bufs=2))
        probs_p = ctx.enter_context(
            tc.tile_pool(name="probs", bufs=int(os.environ.get("K_PRBUFS", "3")))
        )
        asb_p = ctx.enter_context(tc.tile_pool(name="asb", bufs=2))
        den_p = ctx.enter_context(tc.tile_pool(name="den", bufs=2))
        ostage = ctx.enter_context(tc.tile_pool(name="ostage", bufs=2))
        warm_p = ctx.enter_context(tc.tile_pool(name="warm", bufs=1))
        ps_sc = ctx.enter_context(
            tc.tile_pool(
                name="ps_sc", bufs=int(os.environ.get("K_SCBUFS", "2")), space="PSUM"
            )
        )
        ps_acc = ctx.enter_context(tc.tile_pool(name="ps_acc", bufs=1, space="PSUM"))
        ps_io = ctx.enter_context(tc.tile_pool(name="ps_io", bufs=1, space="PSUM"))

        # ---- exp table prefetch: 1-element dummy activation at t~0 ----
        warm = warm_p.tile([1, 8], FP32, tag="warm")
        nc.vector.memset(warm[:, 0:4], 0.0)
        nc.scalar.activation(
            warm[:, 4:8], warm[:, 0:4], mybir.ActivationFunctionType.Exp, scale=1.0
        )
        # ---- PE HAM warm-up: dummy matmuls during the input-DMA wait ----
        # The PE clock-gate (HAM) needs ~3.4us of sustained matmul activity
        # to un-throttle from 1.2 to 2.4 GHz.  The input DMAs take ~11us,
        # so without this the whole prologue runs at half clock.  Dummy
        # N=128 matmuls on a memset tile keep the PE busy until real data
        # lands; they are ahead of the real work in the PE FIFO and finish
        # just before it becomes ready.
        zdum = warm_p.tile([128, 128], BF16, tag="zdum")
        nc.vector.memset(zdum[:], 0.0)
        # ~26 dummies run cold before HAM un-throttles (~3.4us); a few more
        # keep it warm until wq lands (~10.5us).  More than that delays the
        # first real LDWEIGHTS behind the dummy stream.
        n_warm = int(os.environ.get("K_WARM_MM", "40"))
        if n_warm:
            pwarm = ps_sc.tile([128, 128], FP32, tag="sc", name="pwarm")
            for _ in range(n_warm):
                nc.tensor.matmul(
                    pwarm[:],
                    lhsT=zdum[:],
                    rhs=zdum[:],
                    start=True,
                    stop=True,
                    skip_group_check=True,
                )

        # ---- constants / weights / x to SBUF, priority-ordered ----
        # Queue assignment (3 parallel DMA paths): sync=HW, scalar=HW,
        # gpsimd=SW.  Critical prefix: wq; xT j=0; wk; cos/sin j=0.
        xT_sb = consts.tile([128, DQ, T], BF16, tag="xT")
        xT_r = xT_in.rearrange("(o p) m -> p o m", p=128)
        wq_sb = consts.tile([128, DQ, E], BF16, tag="wq")
        wq_r = wqT.rearrange("(o p) m -> p o m", p=128)
        wk_sb = consts.tile([128, DQ, E], BF16, tag="wk")
        wk_r = wkT.rearrange("(o p) m -> p o m", p=128)
        wv_sb = consts.tile([128, DQ, E], BF16, tag="wv")
        wv_r = wvT.rearrange("(o p) m -> p o m", p=128)
        wp_sb = consts.tile([128, 2, D], BF16, tag="wp")
        cos_sb = consts.tile([128, T], BF16, tag="cos")
        sin_sb = consts.tile([128, T], BF16, tag="sin")
        dm_sb = consts.tile([128, 2, HPC, WA], BF16, tag="dmask")

        # wave 0: the prologue's critical inputs, spread across all three
        # DMA queues: wq (sync), xT j0 (all queues), cos/sin j0 (scalar).
        nc.sync.dma_start(wq_sb[:, 0:4, :], wq_r[:, 0:4, :])
        nc.sync.dma_start(wq_sb[:, 4:8, :], wq_r[:, 4:8, :])
        nc.scalar.dma_start(cos_sb[:, ts(0, WP)], cosT[:, ts(0, WP)])
        nc.scalar.dma_start(sin_sb[:, ts(0, WP)], sinT[:, ts(0, WP)])
        for dq in range(3):
            nc.scalar.dma_start(xT_sb[:, dq, ts(0, WP)], xT_r[:, dq, ts(0, WP)])
        for dq in range(3, 6):
            nc.gpsimd.dma_start(xT_sb[:, dq, ts(0, WP)], xT_r[:, dq, ts(0, WP)])
        for dq in range(6, 8):
            nc.sync.dma_start(xT_sb[:, dq, ts(0, WP)], xT_r[:, dq, ts(0, WP)])
        # wave 1: wk, wv, and xT j1 balanced across the queues so the j=1
        # projection quantum (drained during chunk 1) never head-of-line
        # blocks the PE on a straggling slice.
        nc.sync.dma_start(wk_sb[:, 0:4, :], wk_r[:, 0:4, :])
        nc.scalar.dma_start(wk_sb[:, 4:8, :], wk_r[:, 4:8, :])
        nc.gpsimd.dma_start(wv_sb[:, 0:4, :], wv_r[:, 0:4, :])
        nc.gpsimd.dma_start(wv_sb[:, 4:8, :], wv_r[:, 4:8, :])
        for dq in range(3):
            nc.scalar.dma_start(xT_sb[:, dq, ts(1, WP)], xT_r[:, dq, ts(1, WP)])
        for dq in range(3, 6):
            nc.gpsimd.dma_start(xT_sb[:, dq, ts(1, WP)], xT_r[:, dq, ts(1, WP)])
        for dq in range(6, 8):
            nc.sync.dma_start(xT_sb[:, dq, ts(1, WP)], xT_r[:, dq, ts(1, WP)])
        nc.scalar.dma_start(dm_sb[:], dmask[:])
        nc.scalar.dma_start(cos_sb[:, ts(1, WP)], cosT[:, ts(1, WP)])
        nc.scalar.dma_start(sin_sb[:, ts(1, WP)], sinT[:, ts(1, WP)])
        # v_aug[:, t, h, :]: even h = [v | ones], odd h = [ones | v]; fill
        # everything with ones, the v copies overwrite their halves.
        # memset on vector: the DVE is idle until the first projection psum
        # lands (~16us), so this is free there, and it keeps the gpsimd
        # engine clear for SW-DGE descriptor generation.
        v_aug = consts.tile([128, NKT, HPC, 128], BF16, tag="vaug")
        nc.vector.memset(v_aug[:], 1.0)

        # wave 2: wp (gpsimd), xT j2 (sync/gpsimd), cos/sin j2-3 (scalar)
        nc.gpsimd.dma_start(wp_sb[:], wpT.rearrange("(o p) m -> p o m", p=128))
        for dq in range(0, DQ, 2):
            nc.sync.dma_start(xT_sb[:, dq, ts(2, WP)], xT_r[:, dq, ts(2, WP)])
            nc.gpsimd.dma_start(
                xT_sb[:, dq + 1, ts(2, WP)], xT_r[:, dq + 1, ts(2, WP)]
            )
        for j in (2, 3):
            nc.scalar.dma_start(cos_sb[:, ts(j, WP)], cosT[:, ts(j, WP)])
            nc.scalar.dma_start(sin_sb[:, ts(j, WP)], sinT[:, ts(j, WP)])
        for dq in range(0, DQ, 2):
            nc.sync.dma_start(xT_sb[:, dq, ts(3, WP)], xT_r[:, dq, ts(3, WP)])
            nc.gpsimd.dma_start(
                xT_sb[:, dq + 1, ts(3, WP)], xT_r[:, dq + 1, ts(3, WP)]
            )

        # q/k pair tiles [128, T]: head 2p on partitions 0:64, head 2p+1 on
        # 64:128.  The scores matmuls for a pair run as two concurrent
        # 64-row PE tiles (tile_position (0,0) / (64,0)) — both halves of
        # the systolic array stream at once, halving scores wall time.
        q_nat = [
            consts.tile([128, T], BF16, tag=f"qnat{p}", name=f"qnat{p}")
            for p in range(2)
        ]
        k_nat = [
            consts.tile([128, T], BF16, tag=f"knat{p}", name=f"knat{p}")
            for p in range(2)
        ]
        # head h -> sct/pr column slot: pair members go to different psum
        # BANKS (h0/h2 in bank 0 = slots 0,1; h1/h3 in bank 1 = slots 2,3)
        # so the two concurrent streams never write the same bank.
        SLOT = [0, 2, 1, 3]
        attn_nrm = [
            consts.tile([128, T], BF16, tag=f"anrm{p}", name=f"anrm{p}")
            for p in range(2)
        ]

        # ---- work quanta (proj / outproj), drained between attn iters ----
        def emit_qk(j, w_sb, nat, dq_order=None, trickle=False):
            pqk = ps_io.tile([128, 2, WP], FP32, tag="io", name="pqk")
            dqs = dq_order if dq_order is not None else range(DQ)
            # dq-major so each matmul only needs its own xT slice (slices
            # arrive one at a time during the prologue); the interleaved
            # bank accumulation groups are fine on HW.
            for n, dq in enumerate(dqs):
                if trickle and n % 2 == 0:
                    # prologue matmuls stall on arriving xT slices; dummy
                    # LDWEIGHTS between them keep the HAM clock-gate warm
                    nc.tensor.ldweights(weights=zdum[:])
                    nc.tensor.ldweights(weights=zdum[:])
                for half in range(2):
                    nc.tensor.matmul(
                        pqk[:, half, :],
                        lhsT=w_sb[:, dq, ds(128 * half, 128)],
                        rhs=xT_sb[:, dq, ts(j, WP)],
                        start=(n == 0),
                        stop=(n == DQ - 1),
                        skip_group_check=True,
                    )
            lo, hi = pqk[:, 0, :], pqk[:, 1, :]
            cs, sn = cos_sb[:, ts(j, WP)], sin_sb[:, ts(j, WP)]
            st = ropet.tile([128, 2, WP], BF16, tag="st", name="st")
            # bf16 intermediates: the final add/sub then runs all-16-bit
            # operands at 2x DVE rate
            ta = ropet.tile([128, 2, WP], BF16, tag="ta", name="ta")
            tb = ropet.tile([128, 2, WP], BF16, tag="tb", name="tb")
            nc.vector.tensor_mul(ta[:, 0, :], lo, cs)
            nc.vector.tensor_mul(ta[:, 1, :], hi, sn)
            nc.vector.tensor_sub(st[:, 0, :], ta[:, 0, :], ta[:, 1, :])
            nc.vector.tensor_mul(tb[:, 0, :], hi, cs)
            nc.vector.tensor_mul(tb[:, 1, :], lo, sn)
            nc.vector.tensor_add(st[:, 1, :], tb[:, 0, :], tb[:, 1, :])
            # one combined DMA per head: rows land interleaved
            # (lo0,hi0,lo1,hi1,...) — scores are invariant to a row
            # permutation applied consistently to q and k.  Head h goes to
            # pair tile h//2, partition half h%2.
            for h in range(HPC):
                nc.sync.dma_start(
                    nat[h // 2][ds(64 * (h % 2), 64), ts(j, WP)],
                    st[ds(32 * h, 32), :, :],
                )

        def emit_v(j, half_pair):
            pv = ps_io.tile([128, 2, E], FP32, tag="io", name="pv")
            for tt in range(2):
                g = 4 * j + 2 * half_pair + tt
                for dq in range(DQ):
                    nc.tensor.matmul(
                        pv[:, tt, :],
                        lhsT=xT_sb[:, dq, ts(g, 128)],
                        rhs=wv_sb[:, dq, :],
                        start=(dq == 0),
                        stop=(dq == DQ - 1),
                    )
            for tt in range(2):
                g = 4 * j + 2 * half_pair + tt
                for h in range(HPC):
                    voff = 0 if h % 2 == 0 else 64
                    nc.vector.tensor_copy(
                        v_aug[:, g, h, ds(voff, 64)], pv[:, tt, ds(64 * h, 64)]
                    )

        def emit_po(g, pool=None, tag="io", tail=False, hwq=False):
            po = (pool or ps_io).tile([128, D], FP32, tag=tag, name="po")
            if tail:
                # tail tiles pipeline per 512-col half: cast + store of the
                # first half run while the second half's matmuls stream.
                # Stores ride the HW DGE queues (sync/scalar); by the tail
                # the exp stream is finished so scalar is free.
                for dh in range(2):
                    for p in range(2):
                        nc.tensor.matmul(
                            po[:, ds(512 * dh, 512)],
                            lhsT=attn_nrm[p][:, ts(g, 128)],
                            rhs=wp_sb[:, p, ds(512 * dh, 512)],
                            start=(p == 0),
                            stop=(p == 1),
                        )
                    ost = ostage.tile([128, 512], BF16, tag="ost", name="ost")
                    if (g + dh) % 2 == 1:
                        nc.scalar.copy(ost[:], po[:, ds(512 * dh, 512)])
                    else:
                        nc.vector.tensor_copy(ost[:], po[:, ds(512 * dh, 512)])
                    deng = nc.scalar if g % 2 == 1 else nc.sync
                    deng.dma_start(
                        outp[ts(g, 128), ds(512 * dh, 512)], ost[:]
                    )
                return
            for dh in range(2):
                for p in range(2):
                    nc.tensor.matmul(
                        po[:, ds(512 * dh, 512)],
                        lhsT=attn_nrm[p][:, ts(g, 128)],
                        rhs=wp_sb[:, p, ds(512 * dh, 512)],
                        start=(p == 0),
                        stop=(p == 1),
                    )
            ost = ostage.tile([128, D], BF16, tag="ost", name="ost")
            nc.vector.tensor_copy(ost[:], po[:])
            if hwq:
                # near-tail stores ride the HW sync queue; the gpsimd SW
                # queue drains too slowly to sit on the exit path
                nc.sync.dma_start(outp[ts(g, 128), :], ost[:])
            elif g % 2 == 0:
                nc.gpsimd.dma_start(outp[ts(g, 128), :], ost[:])
            else:
                nc.sync.dma_start(outp[ts(g, 128), :], ost[:])

        pending = []
        gap = [0]
        drain_every = [1]

        def drain_one():
            if pending and gap[0] >= drain_every[0]:
                pending.pop(0)()
                gap[0] = 0



        # prologue: projection chunk 0 (serial; nothing to overlap with yet)
        # dq consumption ordered by DMA arrival: gpsimd slices (3,4,5) and
        # scalar slices (0,1,2) land before the sync ones (6,7 behind wq).
        if level >= 1 and not os.environ.get("K_NOPRO"):
            arrival = [3, 4, 5, 0, 1, 2, 6, 7]
            emit_qk(0, wq_sb, q_nat, dq_order=arrival, trickle=True)
            emit_qk(0, wk_sb, k_nat, dq_order=arrival, trickle=True)
            emit_v(0, 0)
            emit_v(0, 1)

        # chunk processing order: chunk 0 second — its inputs are already
        # resident, so it fills the PE while the xT j=1 slices land; chunk 6
        # last so chunk 7's outproj tiles drain during it, leaving only
        # g12,13 for the tail.
        order = [1, 0, 2, 3, 4, 5, 7, 6]
        n_pos = {0: 0, 1: 0, 2: 1, 3: 4, 4: NAC}.get(level, NAC)
        if os.environ.get("K_NCHUNKS"):
            n_pos = int(os.environ["K_NCHUNKS"])
        # ---- attention chunks, with quanta interleaved ----
        for pos in range(n_pos):
            a = order[pos]
            # draining faster than every 2 iters front-loads quanta whose
            # input DMAs haven't landed and head-of-line blocks the PE FIFO
            drain_every[0] = 2
            if level >= 3 and not os.environ.get("K_NOQUANTA"):
                if pos in (0, 2):
                    j = 1 if pos == 0 else 2
                    pending.append(lambda j=j: emit_qk(j, wq_sb, q_nat))
                    pending.append(lambda j=j: emit_qk(j, wk_sb, k_nat))
                    pending.append(lambda j=j: emit_v(j, 0))
                    pending.append(lambda j=j: emit_v(j, 1))
                elif pos == 3:
                    pending.append(lambda: emit_qk(3, wq_sb, q_nat))
                    pending.append(lambda: emit_qk(3, wk_sb, k_nat))
                elif pos == 4:
                    pending.append(lambda: emit_v(3, 0))
                    pending.append(lambda: emit_v(3, 1))
            if level >= 5:
                po_sched = {
                    3: (0, 1),           # chunk 0 (processed at pos 1)
                    4: (2, 3),           # chunk 1
                    5: (4, 5, 6, 7, 8, 9),   # chunks 2,3,4
                    6: (10, 11),         # chunk 5
                    7: (14, 15),         # chunk 7 (processed at pos 6)
                }
                for g in po_sched.get(pos, ()):
                    pending.append(lambda g=g: emit_po(g))

            nk = 2 * a + 2
            asum = ps_acc.tile([128, HPC, WA], FP32, tag="acc", name="asum")

            def S(i, a=a):
                sct = ps_sc.tile([128, HPC, WA], FP32, tag="sc", name="sct")
                # packed pairs: heads 2p / 2p+1 stream concurrently through
                # PE row-groups 0:64 / 64:128 into different psum banks.
                # start=True only on each bank's first writer (h0 -> bank0,
                # h1 -> bank1); the second writer overwrites fresh since the
                # bank's has_written bits were cleared.
                for h in range(HPC):
                    p, half = h // 2, h % 2
                    nc.tensor.matmul(
                        sct[:, SLOT[h], :],
                        lhsT=k_nat[p][ds(64 * half, 64), ts(i, 128)],
                        rhs=q_nat[p][ds(64 * half, 64), ts(a, WA)],
                        start=(h < 2),
                        stop=True,
                        tile_position=(64 * half, 0),
                        skip_group_check=True,
                    )
                return sct

            def EPV(i, sct, nk=nk, asum=asum, a=a):
                pr = probs_p.tile([128, HPC, WA], BF16, tag="pr", name="pr")
                nc.scalar.activation(
                    pr[:], sct[:], mybir.ActivationFunctionType.Exp, scale=SCALE
                )
                if i >= 2 * a:
                    # diagonal k-tile: zero the masked entries on the DVE
                    # (exp(s)*mask == softmax numerator with -inf masking)
                    nc.vector.tensor_mul(pr[:], pr[:], dm_sb[:, i - 2 * a, :, :])
                for h in range(HPC):
                    # i==0, even h: start=True clears the whole bank's
                    # has_written bits; the odd head's first matmul then
                    # overwrites (its bits are clear) — no zero-init needed.
                    nc.tensor.matmul(
                        asum[:, h, :],
                        lhsT=v_aug[:, i, h, :],
                        rhs=pr[:, SLOT[h], :],
                        start=(i == 0 and h % 2 == 0),
                        stop=(i == nk - 1),
                        skip_group_check=True,
                    )

            prev = None
            for i in range(nk):
                sct = S(i)
                if prev is not None:
                    EPV(prev[0], prev[1])
                prev = (i, sct)
                gap[0] += 1
                drain_one()
                if pos < 3:
                    # HAM keep-warm trickle: dummy LDWEIGHTS during the
                    # early dependency gaps keep the PE activity monitor
                    # from re-throttling the clock to 1.2 GHz (two are
                    # needed — one alone measurably fails to hold K=8/8).
                    # Harmless: every real matmul reloads its own weights.
                    # pos 3-4 are quanta-dense: trickles there cost more
                    # inline time than the cold they prevent.
                    nc.tensor.ldweights(weights=zdum[:])
                    nc.tensor.ldweights(weights=zdum[:])
            EPV(prev[0], prev[1])
            # chunk-boundary trickle: the epilogue hand-off can idle the PE
            # past the HAM window; two dummy LDWEIGHTS hold the clock warm
            nc.tensor.ldweights(weights=zdum[:])
            nc.tensor.ldweights(weights=zdum[:])

            if os.environ.get("K_NOEPI"):
                continue
            if pos == n_pos - 1 and not os.environ.get("K_NOFASTEPI"):
                # tail fast path: stage only the DENOMINATOR halves to SBUF
                # (half the copy bytes), swap, reciprocal, and multiply
                # reading the numerators straight from psum — ~2us shorter
                # chain before the last outproj tiles can start.
                sd = asb_p.tile([128, 2, WA], FP32, tag="asb", name="sd")
                den = den_p.tile([128, 2, WA], FP32, tag="den", name="den")
                rc = den_p.tile([128, 2, WA], FP32, tag="rc", name="rc")
                nc.vector.tensor_copy(sd[ds(64, 64), :, :], asum[ds(64, 64), 0:4:2, :])
                nc.scalar.copy(sd[ds(0, 64), :, :], asum[ds(0, 64), 1:4:2, :])
                nc.sync.dma_start(den[ds(0, 64), :, :], sd[ds(64, 64), :, :])
                nc.sync.dma_start(den[ds(64, 64), :, :], sd[ds(0, 64), :, :])
                nc.vector.reciprocal_approx_fast(rc[:], den[:])
                for p in range(2):
                    nc.vector.tensor_mul(
                        attn_nrm[p][ds(0, 64), ts(a, WA)],
                        asum[ds(0, 64), 2 * p, :],
                        rc[ds(0, 64), p, :],
                    )
                    nc.vector.tensor_mul(
                        attn_nrm[p][ds(64, 64), ts(a, WA)],
                        asum[ds(64, 64), 2 * p + 1, :],
                        rc[ds(64, 64), p, :],
                    )
                continue
            # epilogue: drain asum per bank-pair (vector), den swap per pair
            # right behind its copy, reciprocal + normalize per pair — the
            # pair-0 chain completes ~1us earlier than a monolithic drain.
            asb = asb_p.tile([128, HPC, WA], FP32, tag="asb", name="asb")
            den = den_p.tile([128, 2, WA], FP32, tag="den", name="den")
            rc = den_p.tile([128, 2, WA], FP32, tag="rc", name="rc")
            # in the fill phase (and the tail, where the exp stream has
            # ended) the DVE is the bottleneck — scalar takes the asum
            # drains there so the reciprocal+normalize chain starts sooner
            drain_scalar = pos < 3 or pos == n_pos - 1
            if drain_scalar:
                nc.scalar.copy(asb[:, 0:2, :], asum[:, 0:2, :])
            else:
                nc.vector.tensor_copy(asb[:, 0:2, :], asum[:, 0:2, :])
            nc.sync.dma_start(den[ds(0, 64), 0, :], asb[ds(64, 64), 0, :])
            nc.sync.dma_start(den[ds(64, 64), 0, :], asb[ds(0, 64), 1, :])
            if drain_scalar:
                nc.scalar.copy(asb[:, 2:4, :], asum[:, 2:4, :])
            else:
                nc.vector.tensor_copy(asb[:, 2:4, :], asum[:, 2:4, :])
            nc.sync.dma_start(den[ds(0, 64), 1, :], asb[ds(64, 64), 2, :])
            nc.sync.dma_start(den[ds(64, 64), 1, :], asb[ds(0, 64), 3, :])
            for p in range(2):
                nc.vector.reciprocal_approx_fast(rc[:, p, :], den[:, p, :])
                nc.vector.tensor_mul(
                    attn_nrm[p][ds(0, 64), ts(a, WA)],
                    asb[ds(0, 64), 2 * p, :],
                    rc[ds(0, 64), p, :],
                )
                nc.vector.tensor_mul(
                    attn_nrm[p][ds(64, 64), ts(a, WA)],
                    asb[ds(64, 64), 2 * p + 1, :],
                    rc[ds(64, 64), p, :],
                )

        # tail: whatever quanta remain + chunk 6's output tiles
        for f in pending:
            f()
        if level >= 5:
            # tail tiles go in the now-idle scores pool so they run in
            # parallel instead of serializing on the single-buffer io ring
            for g in (12, 13):
                emit_po(g, pool=ps_sc, tag="sc", tail=True)

    nc.compile()
    return nc


def make_consts(cos, sin):
    cosT = np.ascontiguousarray(
        np.tile(np.asarray(cos[0], dtype=np.float32).T[:32], (4, 1))
    ).astype(ml_dtypes.bfloat16)
    sinT = np.ascontiguousarray(
        np.tile(np.asarray(sin[0], dtype=np.float32).T[:32], (4, 1))
    ).astype(ml_dtypes.bfloat16)
    # dmask[kp, idx*HPC*WA + h*WA + qc] = 1 if (128*idx + kp) <= qc else 0
    kp = np.arange(128)[:, None]
    qc = np.arange(WA)[None, :]
    dm = np.stack(
        [
            np.repeat(((128 * idx + kp) <= qc)[:, None, :], HPC, axis=1)
            for idx in range(2)
        ],
        axis=1,
    )  # [128, 2, HPC, WA]
    dmask = np.ascontiguousarray(
        dm.reshape(128, 2 * HPC * WA).astype(ml_dtypes.bfloat16)
    )
    return dict(cosT=cosT, sinT=sinT, dmask=dmask)


def host_prep(core, xT_by_batch, Wq, Wk, Wv, Wp, consts):
    b, hp = core // 4, core % 4
    h0 = hp * HPC
    rows = slice(HD * h0, HD * h0 + E)
    Wq_s = np.asarray(Wq[rows]).reshape(HPC, HD, D)
    Wk_s = np.asarray(Wk[rows]).reshape(HPC, HD, D)
    wqT = np.ascontiguousarray(
        np.concatenate(
            [Wq_s[:, :32].reshape(128, D), Wq_s[:, 32:].reshape(128, D)], 0
        ).T.astype(ml_dtypes.bfloat16)
    )
    wkT = np.ascontiguousarray(
        np.concatenate(
            [Wk_s[:, :32].reshape(128, D), Wk_s[:, 32:].reshape(128, D)], 0
        ).T.astype(ml_dtypes.bfloat16)
    )
    wvT = np.ascontiguousarray(np.asarray(Wv[rows]).T.astype(ml_dtypes.bfloat16))
    wpT = np.ascontiguousarray(np.asarray(Wp[:, rows]).T.astype(ml_dtypes.bfloat16))
    return dict(
        xT_b=xT_by_batch[b],
        wqT=wqT,
        wkT=wkT,
        wvT=wvT,
        wpT=wpT,
        **consts,
    )


_NC_CACHE = None


def _get_nc():
    global _NC_CACHE
    if _NC_CACHE is None:
        _NC_CACHE = build_program()
    return _NC_CACHE


def kernel(x, cos, sin, Wq, Wk, Wv, Wp, _want_trace=False):
    x, cos, sin = np.asarray(x), np.asarray(cos), np.asarray(sin)
    Wq, Wk, Wv, Wp = (np.asarray(a) for a in (Wq, Wk, Wv, Wp))
    nc = _get_nc()
    consts = make_consts(cos, sin)
    xT_by_batch = [
        np.ascontiguousarray(x[b].T.astype(ml_dtypes.bfloat16)) for b in range(B)
    ]
    in_maps = [
        host_prep(core, xT_by_batch, Wq, Wk, Wv, Wp, consts) for core in range(8)
    ]
    res = run_bass_kernel_spmd(nc, in_maps, list(range(8)), trace=_want_trace)
    out = np.zeros((B, T, D), dtype=np.float32)
    for core in range(8):
        out[core // 4] += np.asarray(res.results[core]["outp"], dtype=np.float32)
    if _want_trace:
        kernel.last_exec_time_ns = res.exec_time_ns
        kernel.last_profile = res.profile_json
    return out
